# revision 14
# baseline (speedup 1.0000x reference)
"""Trainium2 Bass kernel for nn_Encoder (tri-modal Mamba encoder), v2.

kernel(**inputs) takes FULL unsharded numpy inputs and returns the FULL
output (B, W, 2N+E, D). Batch B=8 is sharded across 8 NeuronCores (pure
data parallel); params are replicated.

v2 design (per core, one batch element):
- Host pre-packs activations into bf16 feature-major tiles with column
  order (t, q) and 24 zero pad columns per 8-seq tile, so the causal
  depthwise conv becomes 4 time-shifted accumulating PE matmuls with
  plain contiguous operands (shift by d = 8*d columns).
- The (q, t) reorder the scan needs happens inside ACT/DVE ops via
  3D/4D strided output APs (dA exp, u-mult, Crep copy, z silu).
- A[d,s] folds into the dt-expansion weights (deltaA, bf16) so the dA
  exp needs no per-group scale and pairs of groups share 1024-wide ACT
  ops. conv_b/dt_b biases are K=2 rank-2 matmuls so silu/softplus pair
  across the two DI halves.
- LayerNorm affine folds into host-premultiplied weights plus K<=2
  broadcast matmuls; rsqrt = exp(-0.5*ln(var+eps)); Prelu for LeakyReLU;
  only silu blocks switch ACT tables.
- Engines: PE expansions+sums+conv+proj, ACT exps/silus/evacuations,
  DVE u-mult + 2048-wide scans, Pool yh-mults + squares.
- Weights stage via gpsimd cast-DMA (f32 DRAM -> f32r SBUF).
"""

import ml_dtypes
import numpy as np
from contextlib import ExitStack

import concourse.bass as bass
import concourse.tile as tile
from concourse import bacc, mybir
from concourse.bass_utils import run_bass_kernel_spmd

D, DI, SS, KK, RR = 128, 256, 16, 4, 8
B, W, N, E = 8, 64, 128, 256
Q = 128                      # seqs per block
CT = 512                     # columns (tokens) per tile = 8 seqs
TPB = Q * W // CT            # 16 tiles per block
CBLK = Q * W                 # 8192 tokens per block
PADC = 24                    # zero pad columns per tile (3 t-steps * 8 seqs)
TW = PADC + CT               # packed tile width
f32 = mybir.dt.float32
f32r = mybir.dt.float32r
bf16 = mybir.dt.bfloat16
AF = mybir.ActivationFunctionType
OP = mybir.AluOpType
N_CORES = 8
LN_EPS = 1e-5
NW = CBLK // 128             # 64 token-chunks per stat partition

# blocks: (name, modality index, block index in xpack/outp)
BLOCKS = [("n", 0, 0), ("t0", 1, 1), ("t1", 1, 2), ("l", 2, 3)]


class Pack:
    """Column allocator for a (128, *) packed parameter array."""

    def __init__(self, np_dtype):
        self.cols = []
        self.off = {}
        self.n = 0
        self.np_dtype = np_dtype

    def add(self, name, arr):
        arr = np.asarray(arr, np.float64)
        assert arr.ndim == 2 and arr.shape[0] <= 128
        a = np.zeros((128, arr.shape[1]), np.float64)
        a[: arr.shape[0]] = arr
        self.off[name] = (self.n, arr.shape[1])
        self.cols.append(a)
        self.n += arr.shape[1]

    def build(self):
        return np.concatenate(self.cols, axis=1).astype(self.np_dtype)


def _host_pack(inp):
    wp = Pack(np.float32)      # -> f32r on device via gpsimd cast DMA
    bp = Pack(ml_dtypes.bfloat16)
    vp = Pack(np.float32)      # per-partition vectors

    # --- bf16 patterns: sum, delta, deltaA ---
    for g in range(16):
        sm = np.zeros((128, 128))
        for k in range(128):
            sm[k, g * 8 + k // 16] = 1.0
        bp.add(f"sum{g}", sm)
        dl = np.zeros((128, 128))
        for j in range(128):
            dl[g * 8 + j // 16, j] = 1.0
        bp.add(f"delta{g}", dl)
    for m in range(3):
        A = -np.exp(np.asarray(inp["mp_Alog"][m], np.float64))    # (DI, S)
        for cc in range(2):
            for g in range(16):
                dlA = np.zeros((128, 128))
                for j in range(128):
                    dlA[g * 8 + j // 16, j] = A[cc * 128 + g * 8 + j // 16,
                                                j % 16]
                bp.add(f"dA{m}{cc}{g}", dlA)

    # --- f32r weights ---
    for m in range(3):
        win = np.asarray(inp["mp_in"][m], np.float64)              # (D, 2DI)
        cw = np.asarray(inp["mp_conv_w"][m], np.float64)           # (DI, K)
        for cc in range(2):
            sl = slice(cc * 128, (cc + 1) * 128)
            for delta in range(4):                                 # t shift
                k = 3 - delta
                bp.add(f"cx{m}{cc}{delta}", win[:, sl] * cw[sl, k][None, :])
            bp.add(f"z{m}{cc}", win[:, 256 + cc * 128:256 + (cc + 1) * 128])
        wxp = np.asarray(inp["mp_xproj"][m], np.float64)           # (DI, R+2S)
        for cc in range(2):
            blk = wxp[cc * 128:(cc + 1) * 128]
            bp.add(f"wxB{m}{cc}", np.tile(blk[:, RR:RR + SS], (1, 8)))
            bp.add(f"wxC{m}{cc}", np.tile(blk[:, RR + SS:], (1, 8)))
            bp.add(f"wxd{m}{cc}", blk[:, :RR])                     # (128, 8)
        dtw = np.asarray(inp["mp_dt_w"][m], np.float64)            # (R, DI)
        for cc in range(2):
            bp.add(f"dtw{m}{cc}", dtw[:, cc * 128:(cc + 1) * 128])
        wout = np.asarray(inp["mp_out"][m], np.float64)            # (DI, D)
        for cc in range(2):
            bp.add(f"wout{m}{cc}", wout[cc * 128:(cc + 1) * 128])
        ang = np.asarray(inp["an_g"][m], np.float64)
        anb = np.asarray(inp["an_b"][m], np.float64)
        ff1 = np.asarray(inp["ff1_w"][m], np.float64)              # (D, 4D)
        bp.add(f"w1g{m}", ang[:, None] * ff1)                      # LN-A fold
        ff2 = np.asarray(inp["ff2_w"][m], np.float64)              # (4D, D)
        for c4 in range(4):
            bp.add(f"ff2{m}{c4}", ff2[c4 * 128:(c4 + 1) * 128])
        # rank-2 rows (2 partitions used). nm rows carry MINUS sign since
        # the runtime nm tile holds +mean*rstd.
        for cc in range(2):
            bp.add(f"cbrow{m}{cc}",
                   np.asarray(inp["mp_conv_b"][m], np.float64)
                   [None, cc * 128:(cc + 1) * 128])
            bp.add(f"dtbrow{m}{cc}",
                   np.asarray(inp["mp_dt_b"][m], np.float64)
                   [None, cc * 128:(cc + 1) * 128])
        gam1 = ang @ ff1                                           # (4D,)
        bet1 = anb @ ff1 + np.asarray(inp["ff1_b"][m], np.float64)
        for c4 in range(4):
            bp.add(f"gb1row{m}{c4}",
                   np.stack([-gam1[c4 * 128:(c4 + 1) * 128],
                             bet1[c4 * 128:(c4 + 1) * 128]]))
        bp.add(f"af2row{m}", np.stack([-ang,
                                       np.asarray(inp["ff2_b"][m], np.float64)
                                       + anb]))
        bp.add(f"flrow{m}",
               np.stack([-np.asarray(inp["fln_g"][m], np.float64),
                         np.asarray(inp["fln_b"][m], np.float64)]))
    mixw = np.asarray(inp["mix_w"], np.float64)
    for kc in range(2):
        for mc in range(2):
            bp.add(f"mix{kc}{mc}", mixw[kc * 128:(kc + 1) * 128,
                                        mc * 128:(mc + 1) * 128])
    bp.add("ones512", np.ones((1, 512)))
    bp.add("onesD", np.full((128, 1), 1.0 / D))
    bp.add("onescol", np.ones((1, 128)))

    vp.add("eps", np.full((128, 1), LN_EPS))
    for m in range(3):
        for cc in range(2):
            vp.add(f"Dp{m}{cc}", np.asarray(inp["mp_D"][m], np.float64)
                   [cc * 128:(cc + 1) * 128, None])
        vp.add(f"ang{m}", np.asarray(inp["an_g"][m], np.float64)[:, None])
        vp.add(f"flg{m}", np.asarray(inp["fln_g"][m], np.float64)[:, None])
    for mc in range(2):
        vp.add(f"mixb{mc}", np.asarray(inp["mix_b"], np.float64)
               [mc * 128:(mc + 1) * 128, None])
    return wp, vp, bp


def _pack_x(xb):
    """(W, Qs, D) f32 -> (128, ntile*TW) bf16 padded (t,q)-major tiles."""
    Wl, Qs, Dl = xb.shape
    ntile = Qs // 8
    out = np.zeros((128, ntile * TW), np.float32)
    for i in range(ntile):
        sl = xb[:, i * 8:(i + 1) * 8, :]           # (W, 8, D)
        out[:, i * TW + PADC:(i + 1) * TW] = \
            sl.transpose(2, 0, 1).reshape(Dl, W * 8)
    return out.astype(ml_dtypes.bfloat16)


def _emit(ctx, tc, nc, aps, wp, vp, bp):
    wpool = ctx.enter_context(tc.tile_pool(name="weights", bufs=1))
    bw = wpool.tile([128, bp.n], bf16, name="bw")
    nc.sync.dma_start(bw[:], aps["bpack"][:])
    vec = wpool.tile([128, vp.n], f32, name="vec")
    nc.sync.dma_start(vec[:], aps["vpack"][:])
    ones8 = wpool.tile([128, NW], bf16, name="ones8")
    nc.vector.memset(ones8[:], 1.0)

    def BR(name):
        o, c = bp.off[name]
        return bw[:, o:o + c]

    def VP(name):
        o, c = vp.off[name]
        return vec[:, o:o + c]

    def mm(psum_ap, lhsT_ap, rhs_ap, start, stop, kp=None):
        if kp is not None:
            lhsT_ap = lhsT_ap[:kp, :]
            if rhs_ap.partition_size() != kp:
                rhs_ap = rhs_ap[:kp, :]
        nc.tensor.matmul(psum_ap, lhsT_ap, rhs_ap, start=start, stop=stop)

    statA, statF = {}, {}
    for bname, _, _ in BLOCKS:
        statA[bname] = wpool.tile([128, 2 * NW], f32, name=f"sA_{bname}")
        statF[bname] = wpool.tile([128, 2 * NW], f32, name=f"sF_{bname}")

    def tile_stats(res_ap, statT, it, sp, pp, ptag):
        """res (128,512) bf16 SBUF -> mean/meansq sums -> statT chunk it."""
        s2 = sp.tile([128, CT], bf16, name="s2", tag="s2", bufs=2)
        nc.gpsimd.tensor_mul(s2[:], res_ap, res_ap)
        pstb = pp.tile([128, 1024], f32, name="pstb", tag=ptag[0],
                       bufs=ptag[1])
        pst = pstb[0:1, :]
        mm(pst[:, 0:512], BR("onesD"), res_ap, True, True)
        mm(pst[:, 512:1024], BR("onesD"), s2[:], True, True)
        stv = sp.tile([1, 1024], f32, name="stv", tag="stv", bufs=2)
        nc.scalar.activation(stv[:], pst[:], AF.Copy)
        p0 = it * 8
        nc.scalar.dma_start(statT[p0:p0 + 8, 0:NW],
                            stv[:, 0:512].rearrange("x (p w) -> x p w", w=NW))
        nc.scalar.dma_start(statT[p0:p0 + 8, NW:2 * NW],
                            stv[:, 512:1024].rearrange("x (p w) -> x p w",
                                                       w=NW))

    def ln_finish(statT, sp, tag):
        """statT -> bf16 (128, NW) tiles r (rstd) and nm (mean*rstd)."""
        m_t, q_t = statT[:, 0:NW], statT[:, NW:2 * NW]
        var = sp.tile([128, NW], f32, name=f"var{tag}", tag="lnv", bufs=2)
        nc.vector.tensor_mul(var[:], m_t, m_t)
        nc.vector.tensor_sub(var[:], q_t, var[:])
        lg = sp.tile([128, NW], f32, name=f"lg{tag}", tag="lnl", bufs=2)
        nc.scalar.activation(lg[:], var[:], AF.Ln, bias=VP("eps"))
        r_t = sp.tile([128, NW], bf16, name=f"r{tag}", tag=f"lnr{tag}")
        nc.scalar.activation(r_t[:], lg[:], AF.Exp, scale=-0.5)
        nm = sp.tile([128, NW], bf16, name=f"nm{tag}", tag=f"lnm{tag}")
        nc.vector.tensor_mul(nm[:], m_t, r_t[:])
        return r_t, nm

    def load_rst(r_t, nm, it, sp):
        """rr (1, CT) = [r];  rnm (2, CT) = [mean*r; ones]."""
        rr = sp.tile([1, CT], bf16, name="rr", tag="rr", bufs=2)
        rnm = sp.tile([2, CT], bf16, name="rnm", tag="rnm", bufs=2)
        p0 = it * 8
        nc.sync.dma_start(rr[0:1, :].rearrange("x (p w) -> x p w", w=NW),
                          r_t[p0:p0 + 8, :])
        nc.sync.dma_start(rnm[0:1, :].rearrange("x (p w) -> x p w", w=NW),
                          nm[p0:p0 + 8, :])
        nc.sync.dma_start(rnm[1:2, :].rearrange("x (p w) -> x p w", w=NW),
                          ones8[p0:p0 + 8, :])
        return rr, rnm

    # ---------------- phase A: mamba for one tile --------------------
    def phase_a_tile(bname, mi, bi, it, sp, pp, wavep):
        base = (bi * TPB + it) * TW
        xT = sp.tile([128, TW], bf16, name="xT", tag="xT", bufs=2)
        nc.sync.dma_start(xT[:], aps["xpack"][:, base:base + TW])

        # in_proj + conv: pxz (2 banks) used twice: xc pair, then z pair
        pxz1 = pp.tile([128, 1024], f32, name="pxz1", tag="A", bufs=1)
        for cc in range(2):
            for delta in range(4):
                mm(pxz1[:, cc * 512:(cc + 1) * 512],
                   BR(f"cx{mi}{cc}{delta}"),
                   xT[:, PADC - 8 * delta:PADC - 8 * delta + CT],
                   start=(delta == 0), stop=False)
        for cc in range(2):
            mm(pxz1[:, cc * 512:(cc + 1) * 512], BR(f"cbrow{mi}{cc}"),
               BR("ones512"), False, True, kp=1)
        # silu: xc stays (t,q)
        xc = sp.tile([128, 1024], bf16, name="xc", tag="xc", bufs=2)
        nc.scalar.activation(xc[:], pxz1[:], AF.Silu)
        pxz2 = pp.tile([128, 1024], f32, name="pxz2", tag="A", bufs=1)
        for cc in range(2):
            mm(pxz2[:, cc * 512:(cc + 1) * 512],
               BR(f"z{mi}{cc}"), xT[:, PADC:PADC + CT], True, True)
        zg = sp.tile([128, 1024], bf16, name="zg", tag="zg", bufs=2)
        zv = zg[:].rearrange("p (h q t) -> p h q t", h=2, t=W)
        pzv = pxz2[:].rearrange("p (h t q) -> p h t q", h=2, q=8)
        nc.scalar.activation(zv, pzv.rearrange("p h t q -> p h q t"), AF.Silu)

        # x_proj -> [pB | pC]  (tag B, 2 banks)
        pbc = pp.tile([128, 1024], f32, name="pbc", tag="B", bufs=1)
        for cc in range(2):
            mm(pbc[:, 0:512], BR(f"wxB{mi}{cc}"),
               xc[:, cc * 512:(cc + 1) * 512], cc == 0, cc == 1)
            mm(pbc[:, 512:1024], BR(f"wxC{mi}{cc}"),
               xc[:, cc * 512:(cc + 1) * 512], cc == 0, cc == 1)
        Brep = sp.tile([128, CT], bf16, name="Brep", tag="Brep", bufs=2)
        nc.scalar.activation(Brep[:], pbc[:, 0:512], AF.Copy)
        Crep = sp.tile([128, CT], bf16, name="Crep", tag="Crep", bufs=2)
        crv = Crep[:].rearrange("p (q t) -> p q t", t=W)
        pcv = pbc[:, 512:1024].rearrange("p (t q) -> p t q", q=8)
        nc.scalar.activation(crv, pcv.rearrange("p t q -> p q t"), AF.Copy)

        # dt chain reusing tag B
        pd1 = pp.tile([128, 1024], f32, name="pd1", tag="B", bufs=1)
        for cc in range(2):
            mm(pd1[0:8, 0:512], BR(f"wxd{mi}{cc}"),
               xc[:, cc * 512:(cc + 1) * 512], cc == 0, cc == 1)
        dtin = sp.tile([8, CT], bf16, name="dtin", tag="dtin", bufs=2)
        nc.scalar.activation(dtin[:], pd1[0:8, 0:512], AF.Copy)
        pd2 = pp.tile([128, 1024], f32, name="pd2", tag="B", bufs=1)
        for cc in range(2):
            mm(pd2[:, cc * 512:(cc + 1) * 512], BR(f"dtw{mi}{cc}"),
               dtin[:], True, False, kp=8)
        for cc in range(2):
            mm(pd2[:, cc * 512:(cc + 1) * 512], BR(f"dtbrow{mi}{cc}"),
               BR("ones512"), False, True, kp=1)
        ez = sp.tile([128, 1024], bf16, name="ez", tag="ez", bufs=2)
        nc.scalar.activation(ez[:], pd2[:, 0:1024], AF.Exp)
        dtc = sp.tile([128, 1024], bf16, name="dtc", tag="dtc", bufs=2)
        nc.scalar.activation(dtc[:], ez[:], AF.Ln, bias=1.0)
        dtx = sp.tile([128, 1024], bf16, name="dtx", tag="dtx", bufs=2)
        nc.vector.tensor_mul(dtx[:], dtc[:], xc[:])
        # poison t=0 (first 8 cols of each cc half) AFTER dtx
        dpv = dtc[:].rearrange("p (h c) -> p h c", h=2)
        nc.vector.tensor_scalar(dpv[:, :, 0:8], dpv[:, :, 0:8], 0.0, 1.0e4,
                                OP.mult, OP.add)

        # ---- waves: per cc, 4 quads of 4 groups ----
        gated = sp.tile([128, 1024], bf16, name="gated", tag="gated", bufs=2)
        for cc in range(2):
            pY = pp.tile([128, CT], f32, name="pY", tag="py", bufs=1)
            for qd in range(4):
                dA = wavep.tile([128, 2048], bf16, name="dA", tag="dA", bufs=2)
                uu = wavep.tile([128, 2048], bf16, name="uu", tag="uu", bufs=1)
                for half in range(2):
                    g0 = qd * 4 + half * 2
                    for gi in range(2):
                        pP = pp.tile([128, 512], f32, name="pP", tag="P",
                                     bufs=2)
                        mm(pP[:], BR(f"dA{mi}{cc}{g0 + gi}"),
                           dtc[:, cc * 512:(cc + 1) * 512], True, True)
                        dav = dA[:, (half * 2 + gi) * 512:
                                 (half * 2 + gi + 1) * 512].rearrange(
                            "p (q t) -> p q t", t=W)
                        ppv = pP[:].rearrange("p (t q) -> p t q", q=8)
                        nc.scalar.activation(
                            dav, ppv.rearrange("p t q -> p q t"), AF.Exp)
                    for gi in range(2):
                        pX = pp.tile([128, 512], f32, name="pX", tag="P",
                                     bufs=2)
                        mm(pX[:], BR(f"delta{g0 + gi}"),
                           dtx[:, cc * 512:(cc + 1) * 512], True, True)
                        uv = uu[:, (half * 2 + gi) * 512:
                                (half * 2 + gi + 1) * 512].rearrange(
                            "p (q t) -> p q t", t=W)
                        pxv = pX[:].rearrange("p (t q) -> p t q", q=8)
                        brv = Brep[:].rearrange("p (t q) -> p t q", q=8)
                        nc.vector.tensor_mul(
                            uv, pxv.rearrange("p t q -> p q t"),
                            brv.rearrange("p t q -> p q t"))
                hh = wavep.tile([128, 2048], bf16, name="hh", tag="hh", bufs=2)
                nc.vector.tensor_tensor_scan(hh[:], dA[:], uu[:], 0.0,
                                             OP.mult, OP.add)
                yh = wavep.tile([128, 2048], bf16, name="yh", tag="yh", bufs=2)
                crq = Crep[:].rearrange("p (x c) -> p x c", x=1) \
                    .broadcast_to([128, 4, 512])
                nc.gpsimd.tensor_mul(
                    yh[:].rearrange("p (r c) -> p r c", r=4),
                    hh[:].rearrange("p (r c) -> p r c", r=4), crq)
                for gi in range(4):
                    mm(pY[:], BR(f"sum{qd * 4 + gi}"),
                       yh[:, gi * 512:(gi + 1) * 512],
                       qd == 0 and gi == 0, qd == 3 and gi == 3)
            yg = sp.tile([128, CT], bf16, name="yg", tag="yg", bufs=2)
            xcv = xc[:, cc * 512:(cc + 1) * 512].rearrange(
                "p (t q) -> p t q", q=8)
            nc.vector.scalar_tensor_tensor(
                yg[:], xcv.rearrange("p t q -> p q t"), VP(f"Dp{mi}{cc}"),
                pY[:], OP.mult, OP.add)
            nc.vector.tensor_mul(gated[:, cc * 512:(cc + 1) * 512], yg[:],
                                 zg[:, cc * 512:(cc + 1) * 512])
        po = pp.tile([128, CT], f32, name="po", tag="py", bufs=1)
        for cc in range(2):
            mm(po[:], BR(f"wout{mi}{cc}"), gated[:, cc * 512:(cc + 1) * 512],
               cc == 0, cc == 1)
        if bname in ("t0", "t1"):
            res = sp.tile([128, CT], bf16, name="res", tag="res", bufs=2)
            xtv = xT[:, PADC:PADC + CT].rearrange("p (t q) -> p t q", q=8)
            nc.vector.tensor_add(res[:], po[:],
                                 xtv.rearrange("p t q -> p q t"))
            nc.scalar.dma_start(aps[f"scr_s_{bname}"][:, it * CT:(it + 1) * CT],
                                res[:])
            tile_stats(res[:], statA[bname], it, sp, pp, ("A", 1))
        else:
            ym = sp.tile([128, CT], bf16, name="ym", tag="res", bufs=2)
            nc.scalar.activation(ym[:], po[:], AF.Copy)
            nc.scalar.dma_start(aps[f"scr_ym_{bname}"][:, it * CT:(it + 1) * CT],
                                ym[:])

    # ---------------- mix phase (one c0 tile) ------------------------
    def mix_tile(it, sp, pp):
        cat = {}
        for bname in ("n", "l"):
            t_ = sp.tile([128, CT], bf16, name=f"ym{bname}", tag=f"ym{bname}",
                         bufs=2)
            nc.sync.dma_start(t_[:],
                              aps[f"scr_ym_{bname}"][:, it * CT:(it + 1) * CT])
            cat[bname] = t_
        for mc, bname in enumerate(("n", "l")):
            bi = 0 if bname == "n" else 3
            pmxb = pp.tile([128, 1024], f32, name="pmxb", tag="B", bufs=1)
            pmx = pmxb[:, mc * 512:(mc + 1) * 512]
            for kc, bn2 in enumerate(("n", "l")):
                mm(pmx, BR(f"mix{kc}{mc}"), cat[bn2][:], kc == 0, kc == 1)
            ms = sp.tile([128, CT], bf16, name="ms", tag="ms", bufs=2)
            nc.scalar.activation(ms[:], pmx, AF.Silu, bias=VP(f"mixb{mc}"))
            t2 = sp.tile([128, CT], bf16, name="t2m", tag="t2m", bufs=2)
            nc.vector.tensor_add(t2[:], cat[bname][:], ms[:])
            xtile = sp.tile([128, CT], bf16, name="xre", tag="xre", bufs=2)
            base = (bi * TPB + it) * TW
            xtv = aps["xpack"][:, base + PADC:base + TW].rearrange(
                "p (t q) -> p t q", q=8)
            for q8 in range(8):
                nc.sync.dma_start(xtile[:, q8 * W:(q8 + 1) * W],
                                  xtv[:, :, q8])
            res = sp.tile([128, CT], bf16, name="resm", tag="resm", bufs=2)
            nc.vector.tensor_add(res[:], t2[:], xtile[:])
            nc.scalar.dma_start(aps[f"scr_s_{bname}"][:, it * CT:(it + 1) * CT],
                                res[:])
            tile_stats(res[:], statA[bname], it, sp, pp, ("A", 1))

    # ---------------- phase C: LN -> FFN -> LN -> out ----------------
    def phase_c_block(bname, mi, bi, sp, pp):
        rA, nmA = ln_finish(statA[bname], sp, f"A{bname}")

        for it in range(TPB):
            rr, rnm = load_rst(rA, nmA, it, sp)
            sld = sp.tile([128, CT], bf16, name="sld", tag="sld", bufs=2)
            nc.sync.dma_start(sld[:],
                              aps[f"scr_s_{bname}"][:, it * CT:(it + 1) * CT])
            prep = pp.tile([128, CT], f32, name="prep", tag="prep", bufs=2)
            mm(prep[:], BR("onescol"), rr[0:1, :], True, True, kp=1)
            t1 = sp.tile([128, CT], bf16, name="t1", tag="t1", bufs=2)
            nc.vector.tensor_mul(t1[:], sld[:], prep[:])
            hh4 = sp.tile([128, 2048], bf16, name="hh4", tag="hh4", bufs=2)
            for cp in range(2):
                pf = pp.tile([128, 1024], f32, name="pf", tag="pf", bufs=2)
                for ci in range(2):
                    c4 = cp * 2 + ci
                    mm(pf[:, ci * 512:(ci + 1) * 512],
                       BR(f"w1g{mi}")[:, c4 * 128:(c4 + 1) * 128], t1[:],
                       True, False)
                    mm(pf[:, ci * 512:(ci + 1) * 512], BR(f"gb1row{mi}{c4}"),
                       rnm[0:2, :], False, True, kp=2)
                nc.scalar.activation(hh4[:, cp * 1024:(cp + 1) * 1024], pf[:],
                                     AF.Prelu, alpha=0.01)
            pf2 = pp.tile([128, CT], f32, name="pf2", tag="pf2", bufs=2)
            for c4 in range(4):
                mm(pf2[:], BR(f"ff2{mi}{c4}"),
                   hh4[:, c4 * 512:(c4 + 1) * 512], c4 == 0, False)
            mm(pf2[:], BR(f"af2row{mi}"), rnm[0:2, :], False, True, kp=2)
            sft = sp.tile([128, CT], bf16, name="sft", tag="sft", bufs=2)
            nc.vector.scalar_tensor_tensor(sft[:], t1[:], VP(f"ang{mi}"),
                                           pf2[:], OP.mult, OP.add)
            nc.scalar.dma_start(
                aps["scr_sf"][:, bi * CBLK + it * CT:bi * CBLK + (it + 1) * CT],
                sft[:])
            tile_stats(sft[:], statF[bname], it, sp, pp, ("pf", 2))
        rF, nmF = ln_finish(statF[bname], sp, f"F{bname}")
        for it in range(TPB):
            rrF, rnmF = load_rst(rF, nmF, it, sp)
            prepF = pp.tile([128, CT], f32, name="prepF", tag="prep", bufs=2)
            mm(prepF[:], BR("onescol"), rrF[0:1, :], True, True, kp=1)
            sfl = sp.tile([128, CT], bf16, name="sfl", tag="sfl", bufs=2)
            nc.sync.dma_start(
                sfl[:],
                aps["scr_sf"][:, bi * CBLK + it * CT:bi * CBLK + (it + 1) * CT])
            t2 = sp.tile([128, CT], bf16, name="t2c", tag="t2c", bufs=2)
            nc.vector.tensor_mul(t2[:], sfl[:], prepF[:])
            pr2 = pp.tile([128, CT], f32, name="pr2", tag="pf2", bufs=2)
            mm(pr2[:], BR(f"flrow{mi}"), rnmF[0:2, :], True, True, kp=2)
            n2 = sp.tile([128, CT], f32, name="n2", tag="n2", bufs=2)
            nc.vector.scalar_tensor_tensor(n2[:], t2[:], VP(f"flg{mi}"),
                                           pr2[:], OP.mult, OP.add)
            nc.scalar.dma_start(
                aps["outp"][:, bi * CBLK + it * CT:bi * CBLK + (it + 1) * CT],
                n2[:])

    # ------------------------- schedule ------------------------------
    with tc.tile_pool(name="a_sb", bufs=1) as sp, \
            tc.tile_pool(name="a_wv", bufs=1) as wavep:
        with tc.tile_pool(name="a_ps", bufs=1, space="PSUM") as pp:
            for bname, mi, bi in [("n", 0, 0), ("l", 2, 3)]:
                for it in range(TPB):
                    phase_a_tile(bname, mi, bi, it, sp, pp, wavep)
            for it in range(TPB):
                mix_tile(it, sp, pp)
            for bname, mi, bi in [("t0", 1, 1), ("t1", 1, 2)]:
                for it in range(TPB):
                    phase_a_tile(bname, mi, bi, it, sp, pp, wavep)
        with tc.tile_pool(name="c_ps", bufs=1, space="PSUM") as pp2:
            for bname, mi, bi in BLOCKS:
                phase_c_block(bname, mi, bi, sp, pp2)


def _build_program(wp, vp, bp):
    nc = bacc.Bacc("TRN2", target_bir_lowering=False, debug=False,
                   num_devices=N_CORES)
    aps = {}
    aps["xpack"] = nc.dram_tensor("xpack", [128, 4 * TPB * TW], bf16,
                                  kind="ExternalInput").ap()
    aps["vpack"] = nc.dram_tensor("vpack", [128, vp.n], f32,
                                  kind="ExternalInput").ap()
    aps["bpack"] = nc.dram_tensor("bpack", [128, bp.n], bf16,
                                  kind="ExternalInput").ap()
    aps["outp"] = nc.dram_tensor("outp", [128, 4 * CBLK], f32,
                                 kind="ExternalOutput").ap()
    for bname, _, _ in BLOCKS:
        aps[f"scr_s_{bname}"] = nc.dram_tensor(
            f"scr_s_{bname}", [128, CBLK], bf16).ap()
    for bname in ("n", "l"):
        aps[f"scr_ym_{bname}"] = nc.dram_tensor(
            f"scr_ym_{bname}", [128, CBLK], bf16).ap()
    aps["scr_sf"] = nc.dram_tensor("scr_sf", [128, 4 * CBLK], bf16).ap()

    with tile.TileContext(nc) as tc:
        with ExitStack() as ctx:
            _emit(ctx, tc, nc, aps, wp, vp, bp)
    nc.compile()
    return nc


_CACHE = {}


def kernel(**inputs):
    inputs = {k: np.asarray(v) for k, v in inputs.items()}
    wp, vp, bp = _host_pack(inputs)
    if "prog" not in _CACHE:
        _CACHE["prog"] = _build_program(wp, vp, bp)
    nc = _CACHE["prog"]
    vpack, bpack = vp.build(), bp.build()
    in_maps = []
    for b in range(B):
        xp = np.concatenate([
            _pack_x(inputs["x_node"][b]),
            _pack_x(inputs["x_trace"][b][:, 0:128]),
            _pack_x(inputs["x_trace"][b][:, 128:256]),
            _pack_x(inputs["x_log"][b]),
        ], axis=1)
        in_maps.append({"xpack": np.ascontiguousarray(xp),
                        "vpack": vpack, "bpack": bpack})
    res = run_bass_kernel_spmd(nc, in_maps, list(range(N_CORES)))
    out = np.empty((B, W, 2 * N + E, D), np.float32)
    for b in range(B):
        op = res.results[b]["outp"]                     # (128, 4*CBLK)
        for bi, j0 in [(0, 0), (1, N), (2, N + 128), (3, N + E)]:
            blk = op[:, bi * CBLK:(bi + 1) * CBLK]
            arr = blk.reshape(D, TPB, 8, W)             # (D, it, q, t)
            out[b, :, j0:j0 + 128, :] = arr.transpose(3, 1, 2, 0) \
                .reshape(W, 128, D)
    return out


# revision 15
# speedup vs baseline: 1.0586x; 1.0586x over previous
"""Trainium2 Bass kernel for nn_Encoder (tri-modal Mamba encoder), v2.

kernel(**inputs) takes FULL unsharded numpy inputs and returns the FULL
output (B, W, 2N+E, D). Batch B=8 is sharded across 8 NeuronCores (pure
data parallel); params are replicated.

v2 design (per core, one batch element):
- Host pre-packs activations into bf16 feature-major tiles with column
  order (t, q) and 24 zero pad columns per 8-seq tile, so the causal
  depthwise conv becomes 4 time-shifted accumulating PE matmuls with
  plain contiguous operands (shift by d = 8*d columns).
- The (q, t) reorder the scan needs happens inside ACT/DVE ops via
  3D/4D strided output APs (dA exp, u-mult, Crep copy, z silu).
- A[d,s] folds into the dt-expansion weights (deltaA, bf16) so the dA
  exp needs no per-group scale and pairs of groups share 1024-wide ACT
  ops. conv_b/dt_b biases are K=2 rank-2 matmuls so silu/softplus pair
  across the two DI halves.
- LayerNorm affine folds into host-premultiplied weights plus K<=2
  broadcast matmuls; rsqrt = exp(-0.5*ln(var+eps)); Prelu for LeakyReLU;
  only silu blocks switch ACT tables.
- Engines: PE expansions+sums+conv+proj, ACT exps/silus/evacuations,
  DVE u-mult + 2048-wide scans, Pool yh-mults + squares.
- Weights stage via gpsimd cast-DMA (f32 DRAM -> f32r SBUF).
"""

import ml_dtypes
import numpy as np
from contextlib import ExitStack

import concourse.bass as bass
import concourse.tile as tile
from concourse import bacc, mybir
from concourse.bass_utils import run_bass_kernel_spmd

D, DI, SS, KK, RR = 128, 256, 16, 4, 8
B, W, N, E = 8, 64, 128, 256
Q = 128                      # seqs per block
CT = 512                     # columns (tokens) per tile = 8 seqs
TPB = Q * W // CT            # 16 tiles per block
CBLK = Q * W                 # 8192 tokens per block
PADC = 24                    # zero pad columns per tile (3 t-steps * 8 seqs)
TW = PADC + CT               # packed tile width
f32 = mybir.dt.float32
f32r = mybir.dt.float32r
bf16 = mybir.dt.bfloat16
AF = mybir.ActivationFunctionType
OP = mybir.AluOpType
N_CORES = 8
LN_EPS = 1e-5
NW = CBLK // 128             # 64 token-chunks per stat partition

# blocks: (name, modality index, block index in xpack/outp)
BLOCKS = [("n", 0, 0), ("t0", 1, 1), ("t1", 1, 2), ("l", 2, 3)]


class Pack:
    """Column allocator for a (128, *) packed parameter array."""

    def __init__(self, np_dtype):
        self.cols = []
        self.off = {}
        self.n = 0
        self.np_dtype = np_dtype

    def add(self, name, arr):
        arr = np.asarray(arr, np.float64)
        assert arr.ndim == 2 and arr.shape[0] <= 128
        a = np.zeros((128, arr.shape[1]), np.float64)
        a[: arr.shape[0]] = arr
        self.off[name] = (self.n, arr.shape[1])
        self.cols.append(a)
        self.n += arr.shape[1]

    def build(self):
        return np.concatenate(self.cols, axis=1).astype(self.np_dtype)


def _host_pack(inp):
    wp = Pack(np.float32)      # -> f32r on device via gpsimd cast DMA
    bp = Pack(ml_dtypes.bfloat16)
    vp = Pack(np.float32)      # per-partition vectors

    # --- bf16 patterns: sum, delta, deltaA ---
    for g in range(16):
        sm = np.zeros((128, 128))
        for k in range(128):
            sm[k, g * 8 + k // 16] = 1.0
        bp.add(f"sum{g}", sm)
        dl = np.zeros((128, 128))
        for j in range(128):
            dl[g * 8 + j // 16, j] = 1.0
        bp.add(f"delta{g}", dl)
    for m in range(3):
        A = -np.exp(np.asarray(inp["mp_Alog"][m], np.float64))    # (DI, S)
        for cc in range(2):
            for g in range(16):
                dlA = np.zeros((128, 128))
                for j in range(128):
                    dlA[g * 8 + j // 16, j] = A[cc * 128 + g * 8 + j // 16,
                                                j % 16]
                bp.add(f"dA{m}{cc}{g}", dlA)

    # --- f32r weights ---
    for m in range(3):
        win = np.asarray(inp["mp_in"][m], np.float64)              # (D, 2DI)
        cw = np.asarray(inp["mp_conv_w"][m], np.float64)           # (DI, K)
        for cc in range(2):
            sl = slice(cc * 128, (cc + 1) * 128)
            for delta in range(4):                                 # t shift
                k = 3 - delta
                bp.add(f"cx{m}{cc}{delta}", win[:, sl] * cw[sl, k][None, :])
            bp.add(f"z{m}{cc}", win[:, 256 + cc * 128:256 + (cc + 1) * 128])
        wxp = np.asarray(inp["mp_xproj"][m], np.float64)           # (DI, R+2S)
        for cc in range(2):
            blk = wxp[cc * 128:(cc + 1) * 128]
            bp.add(f"wxB{m}{cc}", np.tile(blk[:, RR:RR + SS], (1, 8)))
            bp.add(f"wxC{m}{cc}", np.tile(blk[:, RR + SS:], (1, 8)))
            bp.add(f"wxd{m}{cc}", blk[:, :RR])                     # (128, 8)
        dtw = np.asarray(inp["mp_dt_w"][m], np.float64)            # (R, DI)
        for cc in range(2):
            bp.add(f"dtw{m}{cc}", dtw[:, cc * 128:(cc + 1) * 128])
        wout = np.asarray(inp["mp_out"][m], np.float64)            # (DI, D)
        for cc in range(2):
            bp.add(f"wout{m}{cc}", wout[cc * 128:(cc + 1) * 128])
        ang = np.asarray(inp["an_g"][m], np.float64)
        anb = np.asarray(inp["an_b"][m], np.float64)
        ff1 = np.asarray(inp["ff1_w"][m], np.float64)              # (D, 4D)
        bp.add(f"w1g{m}", ang[:, None] * ff1)                      # LN-A fold
        ff2 = np.asarray(inp["ff2_w"][m], np.float64)              # (4D, D)
        for c4 in range(4):
            bp.add(f"ff2{m}{c4}", ff2[c4 * 128:(c4 + 1) * 128])
        # rank-2 rows (2 partitions used). nm rows carry MINUS sign since
        # the runtime nm tile holds +mean*rstd.
        for cc in range(2):
            bp.add(f"cbrow{m}{cc}",
                   np.asarray(inp["mp_conv_b"][m], np.float64)
                   [None, cc * 128:(cc + 1) * 128])
            bp.add(f"dtbrow{m}{cc}",
                   np.asarray(inp["mp_dt_b"][m], np.float64)
                   [None, cc * 128:(cc + 1) * 128])
        gam1 = ang @ ff1                                           # (4D,)
        bet1 = anb @ ff1 + np.asarray(inp["ff1_b"][m], np.float64)
        for c4 in range(4):
            bp.add(f"gb1row{m}{c4}",
                   np.stack([-gam1[c4 * 128:(c4 + 1) * 128],
                             bet1[c4 * 128:(c4 + 1) * 128]]))
        bp.add(f"af2row{m}", np.stack([-ang,
                                       np.asarray(inp["ff2_b"][m], np.float64)
                                       + anb]))
        bp.add(f"flrow{m}",
               np.stack([-np.asarray(inp["fln_g"][m], np.float64),
                         np.asarray(inp["fln_b"][m], np.float64)]))
    mixw = np.asarray(inp["mix_w"], np.float64)
    for kc in range(2):
        for mc in range(2):
            bp.add(f"mix{kc}{mc}", mixw[kc * 128:(kc + 1) * 128,
                                        mc * 128:(mc + 1) * 128])
    bp.add("ones512", np.ones((1, 512)))
    bp.add("onesD", np.full((128, 1), 1.0 / D))
    bp.add("onescol", np.ones((1, 128)))

    vp.add("eps", np.full((128, 1), LN_EPS))
    for m in range(3):
        for cc in range(2):
            vp.add(f"Dp{m}{cc}", np.asarray(inp["mp_D"][m], np.float64)
                   [cc * 128:(cc + 1) * 128, None])
        vp.add(f"ang{m}", np.asarray(inp["an_g"][m], np.float64)[:, None])
        vp.add(f"flg{m}", np.asarray(inp["fln_g"][m], np.float64)[:, None])
    for mc in range(2):
        vp.add(f"mixb{mc}", np.asarray(inp["mix_b"], np.float64)
               [mc * 128:(mc + 1) * 128, None])
    return wp, vp, bp


def _pack_x(xb):
    """(W, Qs, D) f32 -> (128, ntile*TW) bf16 padded (t,q)-major tiles."""
    Wl, Qs, Dl = xb.shape
    ntile = Qs // 8
    out = np.zeros((128, ntile * TW), np.float32)
    for i in range(ntile):
        sl = xb[:, i * 8:(i + 1) * 8, :]           # (W, 8, D)
        out[:, i * TW + PADC:(i + 1) * TW] = \
            sl.transpose(2, 0, 1).reshape(Dl, W * 8)
    return out.astype(ml_dtypes.bfloat16)


def _emit(ctx, tc, nc, aps, wp, vp, bp):
    wpool = ctx.enter_context(tc.tile_pool(name="weights", bufs=1))
    bw = wpool.tile([128, bp.n], bf16, name="bw")
    nc.sync.dma_start(bw[:], aps["bpack"][:])
    vec = wpool.tile([128, vp.n], f32, name="vec")
    nc.sync.dma_start(vec[:], aps["vpack"][:])
    ones8 = wpool.tile([128, NW], bf16, name="ones8")
    nc.vector.memset(ones8[:], 1.0)

    def BR(name):
        o, c = bp.off[name]
        return bw[:, o:o + c]

    def VP(name):
        o, c = vp.off[name]
        return vec[:, o:o + c]

    def mm(psum_ap, lhsT_ap, rhs_ap, start, stop, kp=None):
        if kp is not None:
            lhsT_ap = lhsT_ap[:kp, :]
            if rhs_ap.partition_size() != kp:
                rhs_ap = rhs_ap[:kp, :]
        nc.tensor.matmul(psum_ap, lhsT_ap, rhs_ap, start=start, stop=stop)

    statA, statF = {}, {}
    for bname, _, _ in BLOCKS:
        statA[bname] = wpool.tile([128, 2 * NW], f32, name=f"sA_{bname}")
        statF[bname] = wpool.tile([128, 2 * NW], f32, name=f"sF_{bname}")

    def tile_stats(res_ap, statT, it, sp, pp, tagm, tagq):
        """res (128,512) bf16 SBUF -> mean/meansq sums -> statT chunk it."""
        s2 = sp.tile([128, CT], bf16, name="s2", tag="s2", bufs=2)
        nc.gpsimd.tensor_mul(s2[:], res_ap, res_ap)
        pm = pp.tile([128, CT], f32, name="pm", tag=tagm[0], bufs=tagm[1])
        mm(pm[0:1, :], BR("onesD"), res_ap, True, True)
        pq = pp.tile([128, CT], f32, name="pq", tag=tagq[0], bufs=tagq[1])
        mm(pq[0:1, :], BR("onesD"), s2[:], True, True)
        stv = sp.tile([1, 1024], f32, name="stv", tag="stv", bufs=2)
        nc.scalar.activation(stv[:, 0:512], pm[0:1, :], AF.Copy)
        nc.scalar.activation(stv[:, 512:1024], pq[0:1, :], AF.Copy)
        p0 = it * 8
        nc.scalar.dma_start(statT[p0:p0 + 8, 0:NW],
                            stv[:, 0:512].rearrange("x (p w) -> x p w", w=NW))
        nc.scalar.dma_start(statT[p0:p0 + 8, NW:2 * NW],
                            stv[:, 512:1024].rearrange("x (p w) -> x p w",
                                                       w=NW))

    def ln_finish(statT, sp, tag):
        """statT -> bf16 (128, NW) tiles r (rstd) and nm (mean*rstd)."""
        m_t, q_t = statT[:, 0:NW], statT[:, NW:2 * NW]
        var = sp.tile([128, NW], f32, name=f"var{tag}", tag="lnv", bufs=2)
        nc.vector.tensor_mul(var[:], m_t, m_t)
        nc.vector.tensor_sub(var[:], q_t, var[:])
        lg = sp.tile([128, NW], f32, name=f"lg{tag}", tag="lnl", bufs=2)
        nc.scalar.activation(lg[:], var[:], AF.Ln, bias=VP("eps"))
        r_t = sp.tile([128, NW], bf16, name=f"r{tag}", tag=f"lnr{tag}")
        nc.scalar.activation(r_t[:], lg[:], AF.Exp, scale=-0.5)
        nm = sp.tile([128, NW], bf16, name=f"nm{tag}", tag=f"lnm{tag}")
        nc.vector.tensor_mul(nm[:], m_t, r_t[:])
        return r_t, nm

    def load_rst(r_t, nm, it, sp):
        """rr (1, CT) = [r];  rnm (2, CT) = [mean*r; ones]."""
        rr = sp.tile([1, CT], bf16, name="rr", tag="rr", bufs=2)
        rnm = sp.tile([2, CT], bf16, name="rnm", tag="rnm", bufs=2)
        p0 = it * 8
        nc.sync.dma_start(rr[0:1, :].rearrange("x (p w) -> x p w", w=NW),
                          r_t[p0:p0 + 8, :])
        nc.sync.dma_start(rnm[0:1, :].rearrange("x (p w) -> x p w", w=NW),
                          nm[p0:p0 + 8, :])
        nc.sync.dma_start(rnm[1:2, :].rearrange("x (p w) -> x p w", w=NW),
                          ones8[p0:p0 + 8, :])
        return rr, rnm

    # ---------------- phase A: mamba for one tile --------------------
    def phase_a_tile(bname, mi, bi, it, sp, pp, wavep):
        base = (bi * TPB + it) * TW
        xT = sp.tile([128, TW], bf16, name="xT", tag="xT", bufs=2)
        nc.sync.dma_start(xT[:], aps["xpack"][:, base:base + TW])

        # in_proj + conv: pxz (2 banks) used twice: xc pair, then z pair
        pxz1 = pp.tile([128, 1024], f32, name="pxz1", tag="A", bufs=1)
        for cc in range(2):
            for delta in range(4):
                mm(pxz1[:, cc * 512:(cc + 1) * 512],
                   BR(f"cx{mi}{cc}{delta}"),
                   xT[:, PADC - 8 * delta:PADC - 8 * delta + CT],
                   start=(delta == 0), stop=False)
        for cc in range(2):
            mm(pxz1[:, cc * 512:(cc + 1) * 512], BR(f"cbrow{mi}{cc}"),
               BR("ones512"), False, True, kp=1)
        # silu: xc stays (t,q)
        xc = sp.tile([128, 1024], bf16, name="xc", tag="xc", bufs=2)
        nc.scalar.activation(xc[:], pxz1[:], AF.Silu)
        pxz2 = pp.tile([128, 1024], f32, name="pxz2", tag="A", bufs=1)
        for cc in range(2):
            mm(pxz2[:, cc * 512:(cc + 1) * 512],
               BR(f"z{mi}{cc}"), xT[:, PADC:PADC + CT], True, True)
        zg = sp.tile([128, 1024], bf16, name="zg", tag="zg", bufs=2)
        zv = zg[:].rearrange("p (h q t) -> p h q t", h=2, t=W)
        pzv = pxz2[:].rearrange("p (h t q) -> p h t q", h=2, q=8)
        nc.scalar.activation(zv, pzv.rearrange("p h t q -> p h q t"), AF.Silu)

        # x_proj -> [pB | pC]  (tag B, 2 banks)
        pbc = pp.tile([128, 1024], f32, name="pbc", tag="B", bufs=1)
        for cc in range(2):
            mm(pbc[:, 0:512], BR(f"wxB{mi}{cc}"),
               xc[:, cc * 512:(cc + 1) * 512], cc == 0, cc == 1)
            mm(pbc[:, 512:1024], BR(f"wxC{mi}{cc}"),
               xc[:, cc * 512:(cc + 1) * 512], cc == 0, cc == 1)
        Brep = sp.tile([128, CT], bf16, name="Brep", tag="Brep", bufs=2)
        nc.scalar.activation(Brep[:], pbc[:, 0:512], AF.Copy)
        Crep = sp.tile([128, CT], bf16, name="Crep", tag="Crep", bufs=2)
        crv = Crep[:].rearrange("p (q t) -> p q t", t=W)
        pcv = pbc[:, 512:1024].rearrange("p (t q) -> p t q", q=8)
        nc.scalar.activation(crv, pcv.rearrange("p t q -> p q t"), AF.Copy)

        # dt chain reusing tag B
        pd1 = pp.tile([128, 1024], f32, name="pd1", tag="B", bufs=1)
        for cc in range(2):
            mm(pd1[0:8, 0:512], BR(f"wxd{mi}{cc}"),
               xc[:, cc * 512:(cc + 1) * 512], cc == 0, cc == 1)
        dtin = sp.tile([8, CT], bf16, name="dtin", tag="dtin", bufs=2)
        nc.scalar.activation(dtin[:], pd1[0:8, 0:512], AF.Copy)
        pd2 = pp.tile([128, 1024], f32, name="pd2", tag="B", bufs=1)
        for cc in range(2):
            mm(pd2[:, cc * 512:(cc + 1) * 512], BR(f"dtw{mi}{cc}"),
               dtin[:], True, False, kp=8)
        for cc in range(2):
            mm(pd2[:, cc * 512:(cc + 1) * 512], BR(f"dtbrow{mi}{cc}"),
               BR("ones512"), False, True, kp=1)
        ez = sp.tile([128, 1024], bf16, name="ez", tag="ez", bufs=2)
        nc.scalar.activation(ez[:], pd2[:, 0:1024], AF.Exp)
        dtc = sp.tile([128, 1024], bf16, name="dtc", tag="dtc", bufs=2)
        nc.scalar.activation(dtc[:], ez[:], AF.Ln, bias=1.0)
        dtx = sp.tile([128, 1024], bf16, name="dtx", tag="dtx", bufs=2)
        nc.vector.tensor_mul(dtx[:], dtc[:], xc[:])
        # poison t=0 (first 8 cols of each cc half) AFTER dtx
        dpv = dtc[:].rearrange("p (h c) -> p h c", h=2)
        nc.vector.tensor_scalar(dpv[:, :, 0:8], dpv[:, :, 0:8], 0.0, 1.0e4,
                                OP.mult, OP.add)

        # ---- waves: per cc, 4 quads of 4 groups ----
        gated = sp.tile([128, 1024], bf16, name="gated", tag="gated", bufs=2)
        for cc in range(2):
            pY = pp.tile([128, CT], f32, name="pY", tag="py", bufs=1)
            for qd in range(4):
                dA = wavep.tile([128, 2048], bf16, name="dA", tag="dA", bufs=2)
                uu = wavep.tile([128, 2048], bf16, name="uu", tag="uu", bufs=1)
                for half in range(2):
                    g0 = qd * 4 + half * 2
                    for gi in range(2):
                        pP = pp.tile([128, 512], f32, name="pP", tag="P",
                                     bufs=2)
                        mm(pP[:], BR(f"dA{mi}{cc}{g0 + gi}"),
                           dtc[:, cc * 512:(cc + 1) * 512], True, True)
                        dav = dA[:, (half * 2 + gi) * 512:
                                 (half * 2 + gi + 1) * 512].rearrange(
                            "p (q t) -> p q t", t=W)
                        ppv = pP[:].rearrange("p (t q) -> p t q", q=8)
                        nc.scalar.activation(
                            dav, ppv.rearrange("p t q -> p q t"), AF.Exp)
                    for gi in range(2):
                        pX = pp.tile([128, 512], f32, name="pX", tag="P",
                                     bufs=2)
                        mm(pX[:], BR(f"delta{g0 + gi}"),
                           dtx[:, cc * 512:(cc + 1) * 512], True, True)
                        uv = uu[:, (half * 2 + gi) * 512:
                                (half * 2 + gi + 1) * 512].rearrange(
                            "p (q t) -> p q t", t=W)
                        pxv = pX[:].rearrange("p (t q) -> p t q", q=8)
                        brv = Brep[:].rearrange("p (t q) -> p t q", q=8)
                        nc.vector.tensor_mul(
                            uv, pxv.rearrange("p t q -> p q t"),
                            brv.rearrange("p t q -> p q t"))
                hh = wavep.tile([128, 2048], bf16, name="hh", tag="hh", bufs=2)
                nc.vector.tensor_tensor_scan(hh[:], dA[:], uu[:], 0.0,
                                             OP.mult, OP.add)
                yh = wavep.tile([128, 2048], bf16, name="yh", tag="yh", bufs=2)
                crq = Crep[:].rearrange("p (x c) -> p x c", x=1) \
                    .broadcast_to([128, 4, 512])
                nc.gpsimd.tensor_mul(
                    yh[:].rearrange("p (r c) -> p r c", r=4),
                    hh[:].rearrange("p (r c) -> p r c", r=4), crq)
                for gi in range(4):
                    mm(pY[:], BR(f"sum{qd * 4 + gi}"),
                       yh[:, gi * 512:(gi + 1) * 512],
                       qd == 0 and gi == 0, qd == 3 and gi == 3)
            yg = sp.tile([128, CT], bf16, name="yg", tag="yg", bufs=2)
            xcv = xc[:, cc * 512:(cc + 1) * 512].rearrange(
                "p (t q) -> p t q", q=8)
            nc.vector.scalar_tensor_tensor(
                yg[:], xcv.rearrange("p t q -> p q t"), VP(f"Dp{mi}{cc}"),
                pY[:], OP.mult, OP.add)
            nc.vector.tensor_mul(gated[:, cc * 512:(cc + 1) * 512], yg[:],
                                 zg[:, cc * 512:(cc + 1) * 512])
        po = pp.tile([128, CT], f32, name="po", tag="py", bufs=1)
        for cc in range(2):
            mm(po[:], BR(f"wout{mi}{cc}"), gated[:, cc * 512:(cc + 1) * 512],
               cc == 0, cc == 1)
        if bname in ("t0", "t1"):
            res = sp.tile([128, CT], bf16, name="res", tag="res", bufs=2)
            xtv = xT[:, PADC:PADC + CT].rearrange("p (t q) -> p t q", q=8)
            nc.vector.tensor_add(res[:], po[:],
                                 xtv.rearrange("p t q -> p q t"))
            nc.scalar.dma_start(aps[f"scr_s_{bname}"][:, it * CT:(it + 1) * CT],
                                res[:])
            tile_stats(res[:], statA[bname], it, sp, pp, ("pst", 1), ("py", 1))
        else:
            ym = sp.tile([128, CT], bf16, name="ym", tag="res", bufs=2)
            nc.scalar.activation(ym[:], po[:], AF.Copy)
            nc.scalar.dma_start(aps[f"scr_ym_{bname}"][:, it * CT:(it + 1) * CT],
                                ym[:])

    # ---------------- mix phase (one c0 tile) ------------------------
    def mix_tile(it, sp, pp):
        cat = {}
        for bname in ("n", "l"):
            t_ = sp.tile([128, CT], bf16, name=f"ym{bname}", tag=f"ym{bname}",
                         bufs=2)
            nc.sync.dma_start(t_[:],
                              aps[f"scr_ym_{bname}"][:, it * CT:(it + 1) * CT])
            cat[bname] = t_
        for mc, bname in enumerate(("n", "l")):
            bi = 0 if bname == "n" else 3
            pmxb = pp.tile([128, 1024], f32, name="pmxb", tag="B", bufs=1)
            pmx = pmxb[:, mc * 512:(mc + 1) * 512]
            for kc, bn2 in enumerate(("n", "l")):
                mm(pmx, BR(f"mix{kc}{mc}"), cat[bn2][:], kc == 0, kc == 1)
            ms = sp.tile([128, CT], bf16, name="ms", tag="ms", bufs=2)
            nc.scalar.activation(ms[:], pmx, AF.Silu, bias=VP(f"mixb{mc}"))
            t2 = sp.tile([128, CT], bf16, name="t2m", tag="t2m", bufs=2)
            nc.vector.tensor_add(t2[:], cat[bname][:], ms[:])
            xtile = sp.tile([128, CT], bf16, name="xre", tag="xre", bufs=2)
            base = (bi * TPB + it) * TW
            xtv = aps["xpack"][:, base + PADC:base + TW].rearrange(
                "p (t q) -> p t q", q=8)
            for q8 in range(8):
                nc.sync.dma_start(xtile[:, q8 * W:(q8 + 1) * W],
                                  xtv[:, :, q8])
            res = sp.tile([128, CT], bf16, name="resm", tag="resm", bufs=2)
            nc.vector.tensor_add(res[:], t2[:], xtile[:])
            nc.scalar.dma_start(aps[f"scr_s_{bname}"][:, it * CT:(it + 1) * CT],
                                res[:])
            tile_stats(res[:], statA[bname], it, sp, pp, ("pst", 1), ("py", 1))

    # ---------------- phase C: LN -> FFN -> LN -> out ----------------
    def phase_c_block(bname, mi, bi, sp, pp):
        rA, nmA = ln_finish(statA[bname], sp, f"A{bname}")

        for it in range(TPB):
            rr, rnm = load_rst(rA, nmA, it, sp)
            sld = sp.tile([128, CT], bf16, name="sld", tag="sld", bufs=2)
            nc.sync.dma_start(sld[:],
                              aps[f"scr_s_{bname}"][:, it * CT:(it + 1) * CT])
            prep = pp.tile([128, CT], f32, name="prep", tag="prep", bufs=2)
            mm(prep[:], BR("onescol"), rr[0:1, :], True, True, kp=1)
            t1 = sp.tile([128, CT], bf16, name="t1", tag="t1", bufs=2)
            nc.vector.tensor_mul(t1[:], sld[:], prep[:])
            hh4 = sp.tile([128, 2048], bf16, name="hh4", tag="hh4", bufs=2)
            for cp in range(2):
                pf = pp.tile([128, 1024], f32, name="pf", tag="pf", bufs=2)
                for ci in range(2):
                    c4 = cp * 2 + ci
                    mm(pf[:, ci * 512:(ci + 1) * 512],
                       BR(f"w1g{mi}")[:, c4 * 128:(c4 + 1) * 128], t1[:],
                       True, False)
                    mm(pf[:, ci * 512:(ci + 1) * 512], BR(f"gb1row{mi}{c4}"),
                       rnm[0:2, :], False, True, kp=2)
                nc.scalar.activation(hh4[:, cp * 1024:(cp + 1) * 1024], pf[:],
                                     AF.Prelu, alpha=0.01)
            pf2 = pp.tile([128, CT], f32, name="pf2", tag="pf2", bufs=2)
            for c4 in range(4):
                mm(pf2[:], BR(f"ff2{mi}{c4}"),
                   hh4[:, c4 * 512:(c4 + 1) * 512], c4 == 0, False)
            mm(pf2[:], BR(f"af2row{mi}"), rnm[0:2, :], False, True, kp=2)
            sft = sp.tile([128, CT], bf16, name="sft", tag="sft", bufs=2)
            nc.vector.scalar_tensor_tensor(sft[:], t1[:], VP(f"ang{mi}"),
                                           pf2[:], OP.mult, OP.add)
            nc.scalar.dma_start(
                aps["scr_sf"][:, bi * CBLK + it * CT:bi * CBLK + (it + 1) * CT],
                sft[:])
            tile_stats(sft[:], statF[bname], it, sp, pp, ("prep", 2), ("pf2", 2))
        rF, nmF = ln_finish(statF[bname], sp, f"F{bname}")
        for it in range(TPB):
            rrF, rnmF = load_rst(rF, nmF, it, sp)
            prepF = pp.tile([128, CT], f32, name="prepF", tag="prep", bufs=2)
            mm(prepF[:], BR("onescol"), rrF[0:1, :], True, True, kp=1)
            sfl = sp.tile([128, CT], bf16, name="sfl", tag="sfl", bufs=2)
            nc.sync.dma_start(
                sfl[:],
                aps["scr_sf"][:, bi * CBLK + it * CT:bi * CBLK + (it + 1) * CT])
            t2 = sp.tile([128, CT], bf16, name="t2c", tag="t2c", bufs=2)
            nc.vector.tensor_mul(t2[:], sfl[:], prepF[:])
            pr2 = pp.tile([128, CT], f32, name="pr2", tag="pf2", bufs=2)
            mm(pr2[:], BR(f"flrow{mi}"), rnmF[0:2, :], True, True, kp=2)
            n2 = sp.tile([128, CT], f32, name="n2", tag="n2", bufs=2)
            nc.vector.scalar_tensor_tensor(n2[:], t2[:], VP(f"flg{mi}"),
                                           pr2[:], OP.mult, OP.add)
            nc.scalar.dma_start(
                aps["outp"][:, bi * CBLK + it * CT:bi * CBLK + (it + 1) * CT],
                n2[:])

    # ------------------------- schedule ------------------------------
    with tc.tile_pool(name="a_sb", bufs=1) as sp, \
            tc.tile_pool(name="a_wv", bufs=1) as wavep:
        with tc.tile_pool(name="a_ps", bufs=1, space="PSUM") as pp:
            for bname, mi, bi in [("n", 0, 0), ("l", 2, 3)]:
                for it in range(TPB):
                    phase_a_tile(bname, mi, bi, it, sp, pp, wavep)
            for it in range(TPB):
                mix_tile(it, sp, pp)
            for bname, mi, bi in [("t0", 1, 1), ("t1", 1, 2)]:
                for it in range(TPB):
                    phase_a_tile(bname, mi, bi, it, sp, pp, wavep)
        with tc.tile_pool(name="c_ps", bufs=1, space="PSUM") as pp2:
            for bname, mi, bi in BLOCKS:
                phase_c_block(bname, mi, bi, sp, pp2)


def _build_program(wp, vp, bp):
    nc = bacc.Bacc("TRN2", target_bir_lowering=False, debug=False,
                   num_devices=N_CORES)
    aps = {}
    aps["xpack"] = nc.dram_tensor("xpack", [128, 4 * TPB * TW], bf16,
                                  kind="ExternalInput").ap()
    aps["vpack"] = nc.dram_tensor("vpack", [128, vp.n], f32,
                                  kind="ExternalInput").ap()
    aps["bpack"] = nc.dram_tensor("bpack", [128, bp.n], bf16,
                                  kind="ExternalInput").ap()
    aps["outp"] = nc.dram_tensor("outp", [128, 4 * CBLK], f32,
                                 kind="ExternalOutput").ap()
    for bname, _, _ in BLOCKS:
        aps[f"scr_s_{bname}"] = nc.dram_tensor(
            f"scr_s_{bname}", [128, CBLK], bf16).ap()
    for bname in ("n", "l"):
        aps[f"scr_ym_{bname}"] = nc.dram_tensor(
            f"scr_ym_{bname}", [128, CBLK], bf16).ap()
    aps["scr_sf"] = nc.dram_tensor("scr_sf", [128, 4 * CBLK], bf16).ap()

    with tile.TileContext(nc) as tc:
        with ExitStack() as ctx:
            _emit(ctx, tc, nc, aps, wp, vp, bp)
    nc.compile()
    return nc


_CACHE = {}


def kernel(**inputs):
    inputs = {k: np.asarray(v) for k, v in inputs.items()}
    wp, vp, bp = _host_pack(inputs)
    if "prog" not in _CACHE:
        _CACHE["prog"] = _build_program(wp, vp, bp)
    nc = _CACHE["prog"]
    vpack, bpack = vp.build(), bp.build()
    in_maps = []
    for b in range(B):
        xp = np.concatenate([
            _pack_x(inputs["x_node"][b]),
            _pack_x(inputs["x_trace"][b][:, 0:128]),
            _pack_x(inputs["x_trace"][b][:, 128:256]),
            _pack_x(inputs["x_log"][b]),
        ], axis=1)
        in_maps.append({"xpack": np.ascontiguousarray(xp),
                        "vpack": vpack, "bpack": bpack})
    res = run_bass_kernel_spmd(nc, in_maps, list(range(N_CORES)))
    out = np.empty((B, W, 2 * N + E, D), np.float32)
    for b in range(B):
        op = res.results[b]["outp"]                     # (128, 4*CBLK)
        for bi, j0 in [(0, 0), (1, N), (2, N + 128), (3, N + E)]:
            blk = op[:, bi * CBLK:(bi + 1) * CBLK]
            arr = blk.reshape(D, TPB, 8, W)             # (D, it, q, t)
            out[b, :, j0:j0 + 128, :] = arr.transpose(3, 1, 2, 0) \
                .reshape(W, 128, D)
    return out


# revision 16
# speedup vs baseline: 1.1981x; 1.1318x over previous
"""Trainium2 Bass kernel for nn_Encoder (tri-modal Mamba encoder), v2.

kernel(**inputs) takes FULL unsharded numpy inputs and returns the FULL
output (B, W, 2N+E, D). Batch B=8 is sharded across 8 NeuronCores (pure
data parallel); params are replicated.

v2 design (per core, one batch element):
- Host pre-packs activations into bf16 feature-major tiles with column
  order (t, q) and 24 zero pad columns per 8-seq tile, so the causal
  depthwise conv becomes 4 time-shifted accumulating PE matmuls with
  plain contiguous operands (shift by d = 8*d columns).
- The (q, t) reorder the scan needs happens inside ACT/DVE ops via
  3D/4D strided output APs (dA exp, u-mult, Crep copy, z silu).
- A[d,s] folds into the dt-expansion weights (deltaA, bf16) so the dA
  exp needs no per-group scale and pairs of groups share 1024-wide ACT
  ops. conv_b/dt_b biases are K=2 rank-2 matmuls so silu/softplus pair
  across the two DI halves.
- LayerNorm affine folds into host-premultiplied weights plus K<=2
  broadcast matmuls; rsqrt = exp(-0.5*ln(var+eps)); Prelu for LeakyReLU;
  only silu blocks switch ACT tables.
- Engines: PE expansions+sums+conv+proj, ACT exps/silus/evacuations,
  DVE u-mult + 2048-wide scans, Pool yh-mults + squares.
- Weights stage via gpsimd cast-DMA (f32 DRAM -> f32r SBUF).
"""

import ml_dtypes
import numpy as np
from contextlib import ExitStack

import concourse.bass as bass
import concourse.tile as tile
from concourse import bacc, mybir
from concourse.bass_utils import run_bass_kernel_spmd

D, DI, SS, KK, RR = 128, 256, 16, 4, 8
B, W, N, E = 8, 64, 128, 256
Q = 128                      # seqs per block
CT = 512                     # columns (tokens) per tile = 8 seqs
TPB = Q * W // CT            # 16 tiles per block
CBLK = Q * W                 # 8192 tokens per block
PADC = 24                    # zero pad columns per tile (3 t-steps * 8 seqs)
TW = PADC + CT               # packed tile width
f32 = mybir.dt.float32
f32r = mybir.dt.float32r
bf16 = mybir.dt.bfloat16
AF = mybir.ActivationFunctionType
OP = mybir.AluOpType
N_CORES = 8
LN_EPS = 1e-5
NW = CBLK // 128             # 64 token-chunks per stat partition

# blocks: (name, modality index, block index in xpack/outp)
BLOCKS = [("n", 0, 0), ("t0", 1, 1), ("t1", 1, 2), ("l", 2, 3)]


class Pack:
    """Column allocator for a (128, *) packed parameter array."""

    def __init__(self, np_dtype):
        self.cols = []
        self.off = {}
        self.n = 0
        self.np_dtype = np_dtype

    def add(self, name, arr):
        arr = np.asarray(arr, np.float64)
        assert arr.ndim == 2 and arr.shape[0] <= 128
        a = np.zeros((128, arr.shape[1]), np.float64)
        a[: arr.shape[0]] = arr
        self.off[name] = (self.n, arr.shape[1])
        self.cols.append(a)
        self.n += arr.shape[1]

    def build(self):
        return np.concatenate(self.cols, axis=1).astype(self.np_dtype)


def _host_pack(inp):
    wp = Pack(np.float32)      # -> f32r on device via gpsimd cast DMA
    bp = Pack(ml_dtypes.bfloat16)
    vp = Pack(np.float32)      # per-partition vectors

    # --- bf16 patterns: sum, delta, deltaA ---
    for g in range(16):
        sm = np.zeros((128, 128))
        for k in range(128):
            sm[k, g * 8 + k // 16] = 1.0
        bp.add(f"sum{g}", sm)
        dl = np.zeros((128, 128))
        for j in range(128):
            dl[g * 8 + j // 16, j] = 1.0
        bp.add(f"delta{g}", dl)
    for m in range(3):
        A = -np.exp(np.asarray(inp["mp_Alog"][m], np.float64))    # (DI, S)
        for cc in range(2):
            for g in range(16):
                dlA = np.zeros((128, 128))
                for j in range(128):
                    dlA[g * 8 + j // 16, j] = A[cc * 128 + g * 8 + j // 16,
                                                j % 16]
                bp.add(f"dA{m}{cc}{g}", dlA)

    # --- f32r weights ---
    for m in range(3):
        win = np.asarray(inp["mp_in"][m], np.float64)              # (D, 2DI)
        cw = np.asarray(inp["mp_conv_w"][m], np.float64)           # (DI, K)
        for cc in range(2):
            sl = slice(cc * 128, (cc + 1) * 128)
            for delta in range(4):                                 # t shift
                k = 3 - delta
                bp.add(f"cx{m}{cc}{delta}", win[:, sl] * cw[sl, k][None, :])
            bp.add(f"z{m}{cc}", win[:, 256 + cc * 128:256 + (cc + 1) * 128])
        wxp = np.asarray(inp["mp_xproj"][m], np.float64)           # (DI, R+2S)
        for cc in range(2):
            blk = wxp[cc * 128:(cc + 1) * 128]
            bp.add(f"wxB{m}{cc}", np.tile(blk[:, RR:RR + SS], (1, 8)))
            bp.add(f"wxC{m}{cc}", np.tile(blk[:, RR + SS:], (1, 8)))
            bp.add(f"wxd{m}{cc}", blk[:, :RR])                     # (128, 8)
        dtw = np.asarray(inp["mp_dt_w"][m], np.float64)            # (R, DI)
        for cc in range(2):
            bp.add(f"dtw{m}{cc}", dtw[:, cc * 128:(cc + 1) * 128])
        wout = np.asarray(inp["mp_out"][m], np.float64)            # (DI, D)
        for cc in range(2):
            bp.add(f"wout{m}{cc}", wout[cc * 128:(cc + 1) * 128])
        ang = np.asarray(inp["an_g"][m], np.float64)
        anb = np.asarray(inp["an_b"][m], np.float64)
        ff1 = np.asarray(inp["ff1_w"][m], np.float64)              # (D, 4D)
        bp.add(f"w1g{m}", ang[:, None] * ff1)                      # LN-A fold
        ff2 = np.asarray(inp["ff2_w"][m], np.float64)              # (4D, D)
        for c4 in range(4):
            bp.add(f"ff2{m}{c4}", ff2[c4 * 128:(c4 + 1) * 128])
        # rank-2 rows (2 partitions used). nm rows carry MINUS sign since
        # the runtime nm tile holds +mean*rstd.
        for cc in range(2):
            bp.add(f"cbrow{m}{cc}",
                   np.asarray(inp["mp_conv_b"][m], np.float64)
                   [None, cc * 128:(cc + 1) * 128])
            bp.add(f"dtbrow{m}{cc}",
                   np.asarray(inp["mp_dt_b"][m], np.float64)
                   [None, cc * 128:(cc + 1) * 128])
        gam1 = ang @ ff1                                           # (4D,)
        bet1 = anb @ ff1 + np.asarray(inp["ff1_b"][m], np.float64)
        for c4 in range(4):
            bp.add(f"gb1row{m}{c4}",
                   np.stack([-gam1[c4 * 128:(c4 + 1) * 128],
                             bet1[c4 * 128:(c4 + 1) * 128]]))
        bp.add(f"af2row{m}", np.stack([-ang,
                                       np.asarray(inp["ff2_b"][m], np.float64)
                                       + anb]))
        bp.add(f"flrow{m}",
               np.stack([-np.asarray(inp["fln_g"][m], np.float64),
                         np.asarray(inp["fln_b"][m], np.float64)]))
    mixw = np.asarray(inp["mix_w"], np.float64)
    for kc in range(2):
        for mc in range(2):
            bp.add(f"mix{kc}{mc}", mixw[kc * 128:(kc + 1) * 128,
                                        mc * 128:(mc + 1) * 128])
    bp.add("ones512", np.ones((1, 512)))
    bp.add("onesD", np.full((128, 1), 1.0 / D))
    bp.add("onescol", np.ones((1, 128)))

    vp.add("eps", np.full((128, 1), LN_EPS))
    for m in range(3):
        for cc in range(2):
            vp.add(f"Dp{m}{cc}", np.asarray(inp["mp_D"][m], np.float64)
                   [cc * 128:(cc + 1) * 128, None])
        vp.add(f"ang{m}", np.asarray(inp["an_g"][m], np.float64)[:, None])
        vp.add(f"flg{m}", np.asarray(inp["fln_g"][m], np.float64)[:, None])
    for mc in range(2):
        vp.add(f"mixb{mc}", np.asarray(inp["mix_b"], np.float64)
               [mc * 128:(mc + 1) * 128, None])
    return wp, vp, bp


def _pack_x_qt(xb):
    """(W, Qs, D) f32 -> (128, Qs*W) bf16, (q,t)-major contiguous."""
    Wl, Qs, Dl = xb.shape
    return np.ascontiguousarray(
        xb.transpose(2, 1, 0).reshape(Dl, Qs * Wl)).astype(ml_dtypes.bfloat16)


def _pack_x(xb):
    """(W, Qs, D) f32 -> (128, ntile*TW) bf16 padded (t,q)-major tiles."""
    Wl, Qs, Dl = xb.shape
    ntile = Qs // 8
    out = np.zeros((128, ntile * TW), np.float32)
    for i in range(ntile):
        sl = xb[:, i * 8:(i + 1) * 8, :]           # (W, 8, D)
        out[:, i * TW + PADC:(i + 1) * TW] = \
            sl.transpose(2, 0, 1).reshape(Dl, W * 8)
    return out.astype(ml_dtypes.bfloat16)


def _emit(ctx, tc, nc, aps, wp, vp, bp):
    wpool = ctx.enter_context(tc.tile_pool(name="weights", bufs=1))
    bw = wpool.tile([128, bp.n], bf16, name="bw")
    nc.sync.dma_start(bw[:], aps["bpack"][:])
    vec = wpool.tile([128, vp.n], f32, name="vec")
    nc.sync.dma_start(vec[:], aps["vpack"][:])
    ones8 = wpool.tile([128, NW], bf16, name="ones8")
    nc.vector.memset(ones8[:], 1.0)

    def BR(name):
        o, c = bp.off[name]
        return bw[:, o:o + c]

    def VP(name):
        o, c = vp.off[name]
        return vec[:, o:o + c]

    def mm(psum_ap, lhsT_ap, rhs_ap, start, stop, kp=None):
        if kp is not None:
            lhsT_ap = lhsT_ap[:kp, :]
            if rhs_ap.partition_size() != kp:
                rhs_ap = rhs_ap[:kp, :]
        nc.tensor.matmul(psum_ap, lhsT_ap, rhs_ap, start=start, stop=stop)

    statA, statF = {}, {}
    for bname, _, _ in BLOCKS:
        statA[bname] = wpool.tile([128, 2 * NW], f32, name=f"sA_{bname}")
        statF[bname] = wpool.tile([128, 2 * NW], f32, name=f"sF_{bname}")

    def tile_stats(res_ap, statT, it, sp, pp, tagm, tagq):
        """res (128,512) bf16 SBUF -> mean/meansq sums -> statT chunk it."""
        s2 = sp.tile([128, CT], bf16, name="s2", tag="s2", bufs=2)
        nc.gpsimd.tensor_mul(s2[:], res_ap, res_ap)
        pm = pp.tile([128, CT], f32, name="pm", tag=tagm[0], bufs=tagm[1])
        mm(pm[0:1, :], BR("onesD"), res_ap, True, True)
        pq = pp.tile([128, CT], f32, name="pq", tag=tagq[0], bufs=tagq[1])
        mm(pq[0:1, :], BR("onesD"), s2[:], True, True)
        stv = sp.tile([1, 1024], f32, name="stv", tag="stv", bufs=2)
        nc.scalar.activation(stv[:, 0:512], pm[0:1, :], AF.Copy)
        nc.scalar.activation(stv[:, 512:1024], pq[0:1, :], AF.Copy)
        p0 = it * 8
        nc.scalar.dma_start(statT[p0:p0 + 8, 0:NW],
                            stv[:, 0:512].rearrange("x (p w) -> x p w", w=NW))
        nc.scalar.dma_start(statT[p0:p0 + 8, NW:2 * NW],
                            stv[:, 512:1024].rearrange("x (p w) -> x p w",
                                                       w=NW))

    def ln_finish(statT, sp, tag):
        """statT -> bf16 (128, NW) tiles r (rstd) and nm (mean*rstd)."""
        m_t, q_t = statT[:, 0:NW], statT[:, NW:2 * NW]
        var = sp.tile([128, NW], f32, name=f"var{tag}", tag="lnv", bufs=2)
        nc.vector.tensor_mul(var[:], m_t, m_t)
        nc.vector.tensor_sub(var[:], q_t, var[:])
        lg = sp.tile([128, NW], f32, name=f"lg{tag}", tag="lnl", bufs=2)
        nc.scalar.activation(lg[:], var[:], AF.Ln, bias=VP("eps"))
        r_t = sp.tile([128, NW], bf16, name=f"r{tag}", tag=f"lnr{tag}")
        nc.scalar.activation(r_t[:], lg[:], AF.Exp, scale=-0.5)
        nm = sp.tile([128, NW], bf16, name=f"nm{tag}", tag=f"lnm{tag}")
        nc.vector.tensor_mul(nm[:], m_t, r_t[:])
        return r_t, nm

    def load_rst(r_t, nm, it, sp):
        """rr (1, CT) = [r];  rnm (2, CT) = [mean*r; ones]."""
        rr = sp.tile([1, CT], bf16, name="rr", tag="rr", bufs=2)
        rnm = sp.tile([2, CT], bf16, name="rnm", tag="rnm", bufs=2)
        p0 = it * 8
        nc.sync.dma_start(rr[0:1, :].rearrange("x (p w) -> x p w", w=NW),
                          r_t[p0:p0 + 8, :])
        nc.sync.dma_start(rnm[0:1, :].rearrange("x (p w) -> x p w", w=NW),
                          nm[p0:p0 + 8, :])
        nc.sync.dma_start(rnm[1:2, :].rearrange("x (p w) -> x p w", w=NW),
                          ones8[p0:p0 + 8, :])
        return rr, rnm

    # ---------------- phase A: mamba for one tile --------------------
    def phase_a_tile(bname, mi, bi, it, sp, pp, wavep):
        base = (bi * TPB + it) * TW
        xT = sp.tile([128, TW], bf16, name="xT", tag="xT", bufs=2)
        nc.sync.dma_start(xT[:], aps["xpack"][:, base:base + TW])

        # in_proj + conv: pxz (2 banks) used twice: xc pair, then z pair
        pxz1 = pp.tile([128, 1024], f32, name="pxz1", tag="A", bufs=1)
        for cc in range(2):
            for delta in range(4):
                mm(pxz1[:, cc * 512:(cc + 1) * 512],
                   BR(f"cx{mi}{cc}{delta}"),
                   xT[:, PADC - 8 * delta:PADC - 8 * delta + CT],
                   start=(delta == 0), stop=False)
        for cc in range(2):
            mm(pxz1[:, cc * 512:(cc + 1) * 512], BR(f"cbrow{mi}{cc}"),
               BR("ones512"), False, True, kp=1)
        # silu: xc stays (t,q)
        xc = sp.tile([128, 1024], bf16, name="xc", tag="xc", bufs=2)
        nc.scalar.activation(xc[:], pxz1[:], AF.Silu)
        pxz2 = pp.tile([128, 1024], f32, name="pxz2", tag="A", bufs=1)
        for cc in range(2):
            mm(pxz2[:, cc * 512:(cc + 1) * 512],
               BR(f"z{mi}{cc}"), xT[:, PADC:PADC + CT], True, True)
        zg = sp.tile([128, 1024], bf16, name="zg", tag="zg", bufs=2)
        zv = zg[:].rearrange("p (h q t) -> p h q t", h=2, t=W)
        pzv = pxz2[:].rearrange("p (h t q) -> p h t q", h=2, q=8)
        nc.scalar.activation(zv, pzv.rearrange("p h t q -> p h q t"), AF.Silu)

        # x_proj -> [pB | pC]  (tag B, 2 banks)
        pbc = pp.tile([128, 1024], f32, name="pbc", tag="B", bufs=1)
        for cc in range(2):
            mm(pbc[:, 0:512], BR(f"wxB{mi}{cc}"),
               xc[:, cc * 512:(cc + 1) * 512], cc == 0, cc == 1)
            mm(pbc[:, 512:1024], BR(f"wxC{mi}{cc}"),
               xc[:, cc * 512:(cc + 1) * 512], cc == 0, cc == 1)
        Brep = sp.tile([128, CT], bf16, name="Brep", tag="Brep", bufs=2)
        nc.scalar.activation(Brep[:], pbc[:, 0:512], AF.Copy)
        Crep = sp.tile([128, CT], bf16, name="Crep", tag="Crep", bufs=2)
        crv = Crep[:].rearrange("p (q t) -> p q t", t=W)
        pcv = pbc[:, 512:1024].rearrange("p (t q) -> p t q", q=8)
        nc.scalar.activation(crv, pcv.rearrange("p t q -> p q t"), AF.Copy)

        # dt chain reusing tag B
        pd1 = pp.tile([128, 1024], f32, name="pd1", tag="B", bufs=1)
        for cc in range(2):
            mm(pd1[0:8, 0:512], BR(f"wxd{mi}{cc}"),
               xc[:, cc * 512:(cc + 1) * 512], cc == 0, cc == 1)
        dtin = sp.tile([8, CT], bf16, name="dtin", tag="dtin", bufs=2)
        nc.scalar.activation(dtin[:], pd1[0:8, 0:512], AF.Copy)
        pd2 = pp.tile([128, 1024], f32, name="pd2", tag="B", bufs=1)
        for cc in range(2):
            mm(pd2[:, cc * 512:(cc + 1) * 512], BR(f"dtw{mi}{cc}"),
               dtin[:], True, False, kp=8)
        for cc in range(2):
            mm(pd2[:, cc * 512:(cc + 1) * 512], BR(f"dtbrow{mi}{cc}"),
               BR("ones512"), False, True, kp=1)
        ez = sp.tile([128, 1024], bf16, name="ez", tag="ez", bufs=2)
        nc.scalar.activation(ez[:], pd2[:, 0:1024], AF.Exp)
        dtc = sp.tile([128, 1024], bf16, name="dtc", tag="dtc", bufs=2)
        nc.scalar.activation(dtc[:], ez[:], AF.Ln, bias=1.0)
        dtx = sp.tile([128, 1024], bf16, name="dtx", tag="dtx", bufs=2)
        nc.vector.tensor_mul(dtx[:], dtc[:], xc[:])
        # poison t=0 (first 8 cols of each cc half) AFTER dtx
        dpv = dtc[:].rearrange("p (h c) -> p h c", h=2)
        nc.vector.tensor_scalar(dpv[:, :, 0:8], dpv[:, :, 0:8], 0.0, 1.0e4,
                                OP.mult, OP.add)

        # ---- waves: per cc, 4 quads of 4 groups ----
        gated = sp.tile([128, 1024], bf16, name="gated", tag="gated", bufs=2)
        for cc in range(2):
            pY = pp.tile([128, CT], f32, name="pY", tag="py", bufs=1)
            for qd in range(4):
                dA = wavep.tile([128, 2048], bf16, name="dA", tag="dA", bufs=2)
                uu = wavep.tile([128, 2048], bf16, name="uu", tag="uu", bufs=1)
                for half in range(2):
                    g0 = qd * 4 + half * 2
                    for gi in range(2):
                        pP = pp.tile([128, 512], f32, name="pP", tag="P",
                                     bufs=2)
                        mm(pP[:], BR(f"dA{mi}{cc}{g0 + gi}"),
                           dtc[:, cc * 512:(cc + 1) * 512], True, True)
                        dav = dA[:, (half * 2 + gi) * 512:
                                 (half * 2 + gi + 1) * 512].rearrange(
                            "p (q t) -> p q t", t=W)
                        ppv = pP[:].rearrange("p (t q) -> p t q", q=8)
                        nc.scalar.activation(
                            dav, ppv.rearrange("p t q -> p q t"), AF.Exp)
                    for gi in range(2):
                        pX = pp.tile([128, 512], f32, name="pX", tag="P",
                                     bufs=2)
                        mm(pX[:], BR(f"delta{g0 + gi}"),
                           dtx[:, cc * 512:(cc + 1) * 512], True, True)
                        uv = uu[:, (half * 2 + gi) * 512:
                                (half * 2 + gi + 1) * 512].rearrange(
                            "p (q t) -> p q t", t=W)
                        pxv = pX[:].rearrange("p (t q) -> p t q", q=8)
                        brv = Brep[:].rearrange("p (t q) -> p t q", q=8)
                        nc.vector.tensor_mul(
                            uv, pxv.rearrange("p t q -> p q t"),
                            brv.rearrange("p t q -> p q t"))
                hh = wavep.tile([128, 2048], bf16, name="hh", tag="hh", bufs=2)
                nc.vector.tensor_tensor_scan(hh[:], dA[:], uu[:], 0.0,
                                             OP.mult, OP.add)
                yh = wavep.tile([128, 2048], bf16, name="yh", tag="yh", bufs=2)
                crq = Crep[:].rearrange("p (x c) -> p x c", x=1) \
                    .broadcast_to([128, 4, 512])
                nc.gpsimd.tensor_mul(
                    yh[:].rearrange("p (r c) -> p r c", r=4),
                    hh[:].rearrange("p (r c) -> p r c", r=4), crq)
                for gi in range(4):
                    mm(pY[:], BR(f"sum{qd * 4 + gi}"),
                       yh[:, gi * 512:(gi + 1) * 512],
                       qd == 0 and gi == 0, qd == 3 and gi == 3)
            yg = sp.tile([128, CT], bf16, name="yg", tag="yg", bufs=2)
            xcv = xc[:, cc * 512:(cc + 1) * 512].rearrange(
                "p (t q) -> p t q", q=8)
            nc.vector.scalar_tensor_tensor(
                yg[:], xcv.rearrange("p t q -> p q t"), VP(f"Dp{mi}{cc}"),
                pY[:], OP.mult, OP.add)
            nc.vector.tensor_mul(gated[:, cc * 512:(cc + 1) * 512], yg[:],
                                 zg[:, cc * 512:(cc + 1) * 512])
        po = pp.tile([128, CT], f32, name="po", tag="py", bufs=1)
        for cc in range(2):
            mm(po[:], BR(f"wout{mi}{cc}"), gated[:, cc * 512:(cc + 1) * 512],
               cc == 0, cc == 1)
        if bname in ("t0", "t1"):
            res = sp.tile([128, CT], bf16, name="res", tag="res", bufs=2)
            xtv = xT[:, PADC:PADC + CT].rearrange("p (t q) -> p t q", q=8)
            nc.vector.tensor_add(res[:], po[:],
                                 xtv.rearrange("p t q -> p q t"))
            nc.scalar.dma_start(aps[f"scr_s_{bname}"][:, it * CT:(it + 1) * CT],
                                res[:])
            tile_stats(res[:], statA[bname], it, sp, pp, ("pst", 1), ("py", 1))
        else:
            ym = sp.tile([128, CT], bf16, name="ym", tag="res", bufs=2)
            nc.scalar.activation(ym[:], po[:], AF.Copy)
            nc.scalar.dma_start(aps[f"scr_ym_{bname}"][:, it * CT:(it + 1) * CT],
                                ym[:])

    # ---------------- mix phase (one c0 tile) ------------------------
    def mix_tile(it, sp, pp):
        cat = {}
        for bname in ("n", "l"):
            t_ = sp.tile([128, CT], bf16, name=f"ym{bname}", tag=f"ym{bname}",
                         bufs=2)
            nc.sync.dma_start(t_[:],
                              aps[f"scr_ym_{bname}"][:, it * CT:(it + 1) * CT])
            cat[bname] = t_
        for mc, bname in enumerate(("n", "l")):
            bi = 0 if bname == "n" else 3
            pmxb = pp.tile([128, 1024], f32, name="pmxb", tag="B", bufs=1)
            pmx = pmxb[:, mc * 512:(mc + 1) * 512]
            for kc, bn2 in enumerate(("n", "l")):
                mm(pmx, BR(f"mix{kc}{mc}"), cat[bn2][:], kc == 0, kc == 1)
            ms = sp.tile([128, CT], bf16, name="ms", tag="ms", bufs=2)
            nc.scalar.activation(ms[:], pmx, AF.Silu, bias=VP(f"mixb{mc}"))
            t2 = sp.tile([128, CT], bf16, name="t2m", tag="t2m", bufs=2)
            nc.vector.tensor_add(t2[:], cat[bname][:], ms[:])
            xtile = sp.tile([128, CT], bf16, name="xre", tag="xre", bufs=2)
            qb = (0 if bname == "n" else 1) * CBLK
            nc.sync.dma_start(xtile[:],
                              aps["xqt"][:, qb + it * CT:qb + (it + 1) * CT])
            res = sp.tile([128, CT], bf16, name="resm", tag="resm", bufs=2)
            nc.vector.tensor_add(res[:], t2[:], xtile[:])
            nc.scalar.dma_start(aps[f"scr_s_{bname}"][:, it * CT:(it + 1) * CT],
                                res[:])
            tile_stats(res[:], statA[bname], it, sp, pp, ("pst", 1), ("py", 1))

    # ---------------- phase C: LN -> FFN -> LN -> out ----------------
    def phase_c_block(bname, mi, bi, sp, pp):
        rA, nmA = ln_finish(statA[bname], sp, f"A{bname}")

        for it in range(TPB):
            rr, rnm = load_rst(rA, nmA, it, sp)
            sld = sp.tile([128, CT], bf16, name="sld", tag="sld", bufs=2)
            nc.sync.dma_start(sld[:],
                              aps[f"scr_s_{bname}"][:, it * CT:(it + 1) * CT])
            prep = pp.tile([128, CT], f32, name="prep", tag="prep", bufs=2)
            mm(prep[:], BR("onescol"), rr[0:1, :], True, True, kp=1)
            t1 = sp.tile([128, CT], bf16, name="t1", tag="t1", bufs=2)
            nc.vector.tensor_mul(t1[:], sld[:], prep[:])
            hh4 = sp.tile([128, 2048], bf16, name="hh4", tag="hh4", bufs=2)
            for cp in range(2):
                pf = pp.tile([128, 1024], f32, name="pf", tag="pf", bufs=2)
                for ci in range(2):
                    c4 = cp * 2 + ci
                    mm(pf[:, ci * 512:(ci + 1) * 512],
                       BR(f"w1g{mi}")[:, c4 * 128:(c4 + 1) * 128], t1[:],
                       True, False)
                    mm(pf[:, ci * 512:(ci + 1) * 512], BR(f"gb1row{mi}{c4}"),
                       rnm[0:2, :], False, True, kp=2)
                nc.scalar.activation(hh4[:, cp * 1024:(cp + 1) * 1024], pf[:],
                                     AF.Prelu, alpha=0.01)
            pf2 = pp.tile([128, CT], f32, name="pf2", tag="pf2", bufs=2)
            for c4 in range(4):
                mm(pf2[:], BR(f"ff2{mi}{c4}"),
                   hh4[:, c4 * 512:(c4 + 1) * 512], c4 == 0, False)
            mm(pf2[:], BR(f"af2row{mi}"), rnm[0:2, :], False, True, kp=2)
            sft = sp.tile([128, CT], bf16, name="sft", tag="sft", bufs=2)
            nc.vector.scalar_tensor_tensor(sft[:], t1[:], VP(f"ang{mi}"),
                                           pf2[:], OP.mult, OP.add)
            nc.scalar.dma_start(
                aps["scr_sf"][:, bi * CBLK + it * CT:bi * CBLK + (it + 1) * CT],
                sft[:])
            tile_stats(sft[:], statF[bname], it, sp, pp, ("prep", 2), ("pf2", 2))
        rF, nmF = ln_finish(statF[bname], sp, f"F{bname}")
        for it in range(TPB):
            rrF, rnmF = load_rst(rF, nmF, it, sp)
            prepF = pp.tile([128, CT], f32, name="prepF", tag="prep", bufs=2)
            mm(prepF[:], BR("onescol"), rrF[0:1, :], True, True, kp=1)
            sfl = sp.tile([128, CT], bf16, name="sfl", tag="sfl", bufs=2)
            nc.sync.dma_start(
                sfl[:],
                aps["scr_sf"][:, bi * CBLK + it * CT:bi * CBLK + (it + 1) * CT])
            t2 = sp.tile([128, CT], bf16, name="t2c", tag="t2c", bufs=2)
            nc.vector.tensor_mul(t2[:], sfl[:], prepF[:])
            pr2 = pp.tile([128, CT], f32, name="pr2", tag="pf2", bufs=2)
            mm(pr2[:], BR(f"flrow{mi}"), rnmF[0:2, :], True, True, kp=2)
            n2 = sp.tile([128, CT], f32, name="n2", tag="n2", bufs=2)
            nc.vector.scalar_tensor_tensor(n2[:], t2[:], VP(f"flg{mi}"),
                                           pr2[:], OP.mult, OP.add)
            nc.scalar.dma_start(
                aps["outp"][:, bi * CBLK + it * CT:bi * CBLK + (it + 1) * CT],
                n2[:])

    # ------------------------- schedule ------------------------------
    with tc.tile_pool(name="a_sb", bufs=1) as sp, \
            tc.tile_pool(name="a_wv", bufs=1) as wavep:
        with tc.tile_pool(name="a_ps", bufs=1, space="PSUM") as pp:
            for bname, mi, bi in [("n", 0, 0), ("l", 2, 3)]:
                for it in range(TPB):
                    phase_a_tile(bname, mi, bi, it, sp, pp, wavep)
            for it in range(TPB):
                mix_tile(it, sp, pp)
            for bname, mi, bi in [("t0", 1, 1), ("t1", 1, 2)]:
                for it in range(TPB):
                    phase_a_tile(bname, mi, bi, it, sp, pp, wavep)
        with tc.tile_pool(name="c_ps", bufs=1, space="PSUM") as pp2:
            for bname, mi, bi in BLOCKS:
                phase_c_block(bname, mi, bi, sp, pp2)


def _build_program(wp, vp, bp):
    nc = bacc.Bacc("TRN2", target_bir_lowering=False, debug=False,
                   num_devices=N_CORES)
    aps = {}
    aps["xpack"] = nc.dram_tensor("xpack", [128, 4 * TPB * TW], bf16,
                                  kind="ExternalInput").ap()
    aps["xqt"] = nc.dram_tensor("xqt", [128, 2 * CBLK], bf16,
                                kind="ExternalInput").ap()
    aps["vpack"] = nc.dram_tensor("vpack", [128, vp.n], f32,
                                  kind="ExternalInput").ap()
    aps["bpack"] = nc.dram_tensor("bpack", [128, bp.n], bf16,
                                  kind="ExternalInput").ap()
    aps["outp"] = nc.dram_tensor("outp", [128, 4 * CBLK], f32,
                                 kind="ExternalOutput").ap()
    for bname, _, _ in BLOCKS:
        aps[f"scr_s_{bname}"] = nc.dram_tensor(
            f"scr_s_{bname}", [128, CBLK], bf16).ap()
    for bname in ("n", "l"):
        aps[f"scr_ym_{bname}"] = nc.dram_tensor(
            f"scr_ym_{bname}", [128, CBLK], bf16).ap()
    aps["scr_sf"] = nc.dram_tensor("scr_sf", [128, 4 * CBLK], bf16).ap()

    with tile.TileContext(nc) as tc:
        with ExitStack() as ctx:
            _emit(ctx, tc, nc, aps, wp, vp, bp)
    nc.compile()
    return nc


_CACHE = {}


def kernel(**inputs):
    inputs = {k: np.asarray(v) for k, v in inputs.items()}
    wp, vp, bp = _host_pack(inputs)
    if "prog" not in _CACHE:
        _CACHE["prog"] = _build_program(wp, vp, bp)
    nc = _CACHE["prog"]
    vpack, bpack = vp.build(), bp.build()
    in_maps = []
    for b in range(B):
        xp = np.concatenate([
            _pack_x(inputs["x_node"][b]),
            _pack_x(inputs["x_trace"][b][:, 0:128]),
            _pack_x(inputs["x_trace"][b][:, 128:256]),
            _pack_x(inputs["x_log"][b]),
        ], axis=1)
        xqt = np.concatenate([_pack_x_qt(inputs["x_node"][b]),
                              _pack_x_qt(inputs["x_log"][b])], axis=1)
        in_maps.append({"xpack": np.ascontiguousarray(xp), "xqt": xqt,
                        "vpack": vpack, "bpack": bpack})
    res = run_bass_kernel_spmd(nc, in_maps, list(range(N_CORES)))
    out = np.empty((B, W, 2 * N + E, D), np.float32)
    for b in range(B):
        op = res.results[b]["outp"]                     # (128, 4*CBLK)
        for bi, j0 in [(0, 0), (1, N), (2, N + 128), (3, N + E)]:
            blk = op[:, bi * CBLK:(bi + 1) * CBLK]
            arr = blk.reshape(D, TPB, 8, W)             # (D, it, q, t)
            out[b, :, j0:j0 + 128, :] = arr.transpose(3, 1, 2, 0) \
                .reshape(W, 128, D)
    return out


# revision 17
# speedup vs baseline: 1.6217x; 1.3536x over previous
"""Trainium2 Bass kernel for nn_Encoder (tri-modal Mamba encoder), v2.

kernel(**inputs) takes FULL unsharded numpy inputs and returns the FULL
output (B, W, 2N+E, D). Batch B=8 is sharded across 8 NeuronCores (pure
data parallel); params are replicated.

v2 design (per core, one batch element):
- Host pre-packs activations into bf16 feature-major tiles with column
  order (t, q) and 24 zero pad columns per 8-seq tile, so the causal
  depthwise conv becomes 4 time-shifted accumulating PE matmuls with
  plain contiguous operands (shift by d = 8*d columns).
- The (q, t) reorder the scan needs happens inside ACT/DVE ops via
  3D/4D strided output APs (dA exp, u-mult, Crep copy, z silu).
- A[d,s] folds into the dt-expansion weights (deltaA, bf16) so the dA
  exp needs no per-group scale and pairs of groups share 1024-wide ACT
  ops. conv_b/dt_b biases are K=2 rank-2 matmuls so silu/softplus pair
  across the two DI halves.
- LayerNorm affine folds into host-premultiplied weights plus K<=2
  broadcast matmuls; rsqrt = exp(-0.5*ln(var+eps)); Prelu for LeakyReLU;
  only silu blocks switch ACT tables.
- Engines: PE expansions+sums+conv+proj, ACT exps/silus/evacuations,
  DVE u-mult + 2048-wide scans, Pool yh-mults + squares.
- Weights stage via gpsimd cast-DMA (f32 DRAM -> f32r SBUF).
"""

import ml_dtypes
import numpy as np
from contextlib import ExitStack

import concourse.bass as bass
import concourse.tile as tile
from concourse import bacc, mybir
from concourse.bass_utils import run_bass_kernel_spmd

D, DI, SS, KK, RR = 128, 256, 16, 4, 8
B, W, N, E = 8, 64, 128, 256
Q = 128                      # seqs per block
CT = 512                     # columns (tokens) per tile = 8 seqs
TPB = Q * W // CT            # 16 tiles per block
CBLK = Q * W                 # 8192 tokens per block
PADC = 24                    # zero pad columns per tile (3 t-steps * 8 seqs)
TW = PADC + CT               # packed tile width
f32 = mybir.dt.float32
f32r = mybir.dt.float32r
bf16 = mybir.dt.bfloat16
AF = mybir.ActivationFunctionType
OP = mybir.AluOpType
N_CORES = 8
LN_EPS = 1e-5
NW = CBLK // 128             # 64 token-chunks per stat partition

# blocks: (name, modality index, block index in xpack/outp)
BLOCKS = [("n", 0, 0), ("t0", 1, 1), ("t1", 1, 2), ("l", 2, 3)]


class Pack:
    """Column allocator for a (128, *) packed parameter array."""

    def __init__(self, np_dtype):
        self.cols = []
        self.off = {}
        self.n = 0
        self.np_dtype = np_dtype

    def add(self, name, arr):
        arr = np.asarray(arr, np.float64)
        assert arr.ndim == 2 and arr.shape[0] <= 128
        a = np.zeros((128, arr.shape[1]), np.float64)
        a[: arr.shape[0]] = arr
        self.off[name] = (self.n, arr.shape[1])
        self.cols.append(a)
        self.n += arr.shape[1]

    def build(self):
        return np.concatenate(self.cols, axis=1).astype(self.np_dtype)


def _host_pack(inp):
    wp = Pack(np.float32)      # -> f32r on device via gpsimd cast DMA
    bp = Pack(ml_dtypes.bfloat16)
    vp = Pack(np.float32)      # per-partition vectors

    # --- bf16 patterns: sum, delta, deltaA ---
    for g in range(16):
        sm = np.zeros((128, 128))
        for k in range(128):
            sm[k, g * 8 + k // 16] = 1.0
        bp.add(f"sum{g}", sm)
        dl = np.zeros((128, 128))
        for j in range(128):
            dl[g * 8 + j // 16, j] = 1.0
        bp.add(f"delta{g}", dl)
    for m in range(3):
        A = -np.exp(np.asarray(inp["mp_Alog"][m], np.float64))    # (DI, S)
        for cc in range(2):
            for g in range(16):
                dlA = np.zeros((128, 128))
                for j in range(128):
                    dlA[g * 8 + j // 16, j] = A[cc * 128 + g * 8 + j // 16,
                                                j % 16]
                bp.add(f"dA{m}{cc}{g}", dlA)

    # --- f32r weights ---
    for m in range(3):
        win = np.asarray(inp["mp_in"][m], np.float64)              # (D, 2DI)
        cw = np.asarray(inp["mp_conv_w"][m], np.float64)           # (DI, K)
        for cc in range(2):
            sl = slice(cc * 128, (cc + 1) * 128)
            for delta in range(4):                                 # t shift
                k = 3 - delta
                bp.add(f"cx{m}{cc}{delta}", win[:, sl] * cw[sl, k][None, :])
            bp.add(f"z{m}{cc}", win[:, 256 + cc * 128:256 + (cc + 1) * 128])
        wxp = np.asarray(inp["mp_xproj"][m], np.float64)           # (DI, R+2S)
        for cc in range(2):
            blk = wxp[cc * 128:(cc + 1) * 128]
            bp.add(f"wxB{m}{cc}", np.tile(blk[:, RR:RR + SS], (1, 8)))
            bp.add(f"wxC{m}{cc}", np.tile(blk[:, RR + SS:], (1, 8)))
            bp.add(f"wxd{m}{cc}", blk[:, :RR])                     # (128, 8)
        dtw = np.asarray(inp["mp_dt_w"][m], np.float64)            # (R, DI)
        for cc in range(2):
            bp.add(f"dtw{m}{cc}", dtw[:, cc * 128:(cc + 1) * 128])
        wout = np.asarray(inp["mp_out"][m], np.float64)            # (DI, D)
        for cc in range(2):
            bp.add(f"wout{m}{cc}", wout[cc * 128:(cc + 1) * 128])
        ang = np.asarray(inp["an_g"][m], np.float64)
        anb = np.asarray(inp["an_b"][m], np.float64)
        ff1 = np.asarray(inp["ff1_w"][m], np.float64)              # (D, 4D)
        bp.add(f"w1g{m}", ang[:, None] * ff1)                      # LN-A fold
        ff2 = np.asarray(inp["ff2_w"][m], np.float64)              # (4D, D)
        for c4 in range(4):
            bp.add(f"ff2{m}{c4}", ff2[c4 * 128:(c4 + 1) * 128])
        # rank-2 rows (2 partitions used). nm rows carry MINUS sign since
        # the runtime nm tile holds +mean*rstd.
        for cc in range(2):
            bp.add(f"cbrow{m}{cc}",
                   np.asarray(inp["mp_conv_b"][m], np.float64)
                   [None, cc * 128:(cc + 1) * 128])
            bp.add(f"dtbrow{m}{cc}",
                   np.asarray(inp["mp_dt_b"][m], np.float64)
                   [None, cc * 128:(cc + 1) * 128])
        gam1 = ang @ ff1                                           # (4D,)
        bet1 = anb @ ff1 + np.asarray(inp["ff1_b"][m], np.float64)
        for c4 in range(4):
            bp.add(f"gb1row{m}{c4}",
                   np.stack([-gam1[c4 * 128:(c4 + 1) * 128],
                             bet1[c4 * 128:(c4 + 1) * 128]]))
        bp.add(f"af2row{m}", np.stack([-ang,
                                       np.asarray(inp["ff2_b"][m], np.float64)
                                       + anb]))
        bp.add(f"flrow{m}",
               np.stack([-np.asarray(inp["fln_g"][m], np.float64),
                         np.asarray(inp["fln_b"][m], np.float64)]))
    mixw = np.asarray(inp["mix_w"], np.float64)
    for kc in range(2):
        for mc in range(2):
            bp.add(f"mix{kc}{mc}", mixw[kc * 128:(kc + 1) * 128,
                                        mc * 128:(mc + 1) * 128])
    bp.add("ones512", np.ones((1, 512)))
    bp.add("onesD", np.full((128, 1), 1.0 / D))
    bp.add("onescol", np.ones((1, 128)))

    vp.add("eps", np.full((128, 1), LN_EPS))
    for m in range(3):
        for cc in range(2):
            vp.add(f"Dp{m}{cc}", np.asarray(inp["mp_D"][m], np.float64)
                   [cc * 128:(cc + 1) * 128, None])
        vp.add(f"ang{m}", np.asarray(inp["an_g"][m], np.float64)[:, None])
        vp.add(f"flg{m}", np.asarray(inp["fln_g"][m], np.float64)[:, None])
    for mc in range(2):
        vp.add(f"mixb{mc}", np.asarray(inp["mix_b"], np.float64)
               [mc * 128:(mc + 1) * 128, None])
    return wp, vp, bp


def _pack_x_qt(xb):
    """(W, Qs, D) f32 -> (128, Qs*W) bf16, (q,t)-major contiguous."""
    Wl, Qs, Dl = xb.shape
    return np.ascontiguousarray(
        xb.transpose(2, 1, 0).reshape(Dl, Qs * Wl)).astype(ml_dtypes.bfloat16)


def _pack_x(xb):
    """(W, Qs, D) f32 -> (128, ntile*TW) bf16 padded (t,q)-major tiles."""
    Wl, Qs, Dl = xb.shape
    ntile = Qs // 8
    out = np.zeros((128, ntile * TW), np.float32)
    for i in range(ntile):
        sl = xb[:, i * 8:(i + 1) * 8, :]           # (W, 8, D)
        out[:, i * TW + PADC:(i + 1) * TW] = \
            sl.transpose(2, 0, 1).reshape(Dl, W * 8)
    return out.astype(ml_dtypes.bfloat16)


def _emit(ctx, tc, nc, aps, wp, vp, bp):
    wpool = ctx.enter_context(tc.tile_pool(name="weights", bufs=1))
    bw = wpool.tile([128, bp.n], bf16, name="bw")
    nc.sync.dma_start(bw[:], aps["bpack"][:])
    vec = wpool.tile([128, vp.n], f32, name="vec")
    nc.sync.dma_start(vec[:], aps["vpack"][:])
    ones8 = wpool.tile([128, NW], bf16, name="ones8")
    nc.vector.memset(ones8[:], 1.0)

    def BR(name):
        o, c = bp.off[name]
        return bw[:, o:o + c]

    def VP(name):
        o, c = vp.off[name]
        return vec[:, o:o + c]

    def mm(psum_ap, lhsT_ap, rhs_ap, start, stop, kp=None):
        if kp is not None:
            lhsT_ap = lhsT_ap[:kp, :]
            if rhs_ap.partition_size() != kp:
                rhs_ap = rhs_ap[:kp, :]
        nc.tensor.matmul(psum_ap, lhsT_ap, rhs_ap, start=start, stop=stop)

    statA, statF = {}, {}
    for bname, _, _ in BLOCKS:
        statA[bname] = wpool.tile([128, 2 * NW], f32, name=f"sA_{bname}")
        statF[bname] = wpool.tile([128, 2 * NW], f32, name=f"sF_{bname}")

    def tile_stats(res_ap, statT, it, sp, pp, tagm, tagq):
        """res (128,512) bf16 SBUF -> mean/meansq sums -> statT chunk it."""
        s2 = sp.tile([128, CT], bf16, name="s2", tag="s2", bufs=2)
        nc.gpsimd.tensor_mul(s2[:], res_ap, res_ap)
        pm = pp.tile([128, CT], f32, name="pm", tag=tagm[0], bufs=tagm[1])
        mm(pm[0:1, :], BR("onesD"), res_ap, True, True)
        pq = pp.tile([128, CT], f32, name="pq", tag=tagq[0], bufs=tagq[1])
        mm(pq[0:1, :], BR("onesD"), s2[:], True, True)
        stv = sp.tile([1, 1024], f32, name="stv", tag="stv", bufs=2)
        nc.scalar.activation(stv[:, 0:512], pm[0:1, :], AF.Copy)
        nc.scalar.activation(stv[:, 512:1024], pq[0:1, :], AF.Copy)
        p0 = it * 8
        nc.scalar.dma_start(statT[p0:p0 + 8, 0:NW],
                            stv[:, 0:512].rearrange("x (p w) -> x p w", w=NW))
        nc.scalar.dma_start(statT[p0:p0 + 8, NW:2 * NW],
                            stv[:, 512:1024].rearrange("x (p w) -> x p w",
                                                       w=NW))

    def ln_finish(statT, sp, tag):
        """statT -> bf16 (128, NW) tiles r (rstd) and nm (mean*rstd)."""
        m_t, q_t = statT[:, 0:NW], statT[:, NW:2 * NW]
        var = sp.tile([128, NW], f32, name=f"var{tag}", tag="lnv", bufs=2)
        nc.vector.tensor_mul(var[:], m_t, m_t)
        nc.vector.tensor_sub(var[:], q_t, var[:])
        lg = sp.tile([128, NW], f32, name=f"lg{tag}", tag="lnl", bufs=2)
        nc.scalar.activation(lg[:], var[:], AF.Ln, bias=VP("eps"))
        r_t = sp.tile([128, NW], bf16, name=f"r{tag}", tag=f"lnr{tag}")
        nc.scalar.activation(r_t[:], lg[:], AF.Exp, scale=-0.5)
        nm = sp.tile([128, NW], bf16, name=f"nm{tag}", tag=f"lnm{tag}")
        nc.vector.tensor_mul(nm[:], m_t, r_t[:])
        return r_t, nm

    def load_rst(r_t, nm, it, sp):
        """rr (1, CT) = [r];  rnm (2, CT) = [mean*r; ones]."""
        rr = sp.tile([1, CT], bf16, name="rr", tag="rr", bufs=2)
        rnm = sp.tile([2, CT], bf16, name="rnm", tag="rnm", bufs=2)
        p0 = it * 8
        nc.sync.dma_start(rr[0:1, :].rearrange("x (p w) -> x p w", w=NW),
                          r_t[p0:p0 + 8, :])
        nc.sync.dma_start(rnm[0:1, :].rearrange("x (p w) -> x p w", w=NW),
                          nm[p0:p0 + 8, :])
        nc.sync.dma_start(rnm[1:2, :].rearrange("x (p w) -> x p w", w=NW),
                          ones8[p0:p0 + 8, :])
        return rr, rnm

    # ---------------- phase A: mamba for one tile --------------------
    def phase_a_tile(bname, mi, bi, it, sp, pp, wavep):
        base = (bi * TPB + it) * TW
        xT = sp.tile([128, TW], bf16, name="xT", tag="xT", bufs=2)
        nc.sync.dma_start(xT[:], aps["xpack"][:, base:base + TW])

        # in_proj + conv: pxz (2 banks) used twice: xc pair, then z pair
        pxz1 = pp.tile([128, 1024], f32, name="pxz1", tag="A", bufs=1)
        for cc in range(2):
            for delta in range(4):
                mm(pxz1[:, cc * 512:(cc + 1) * 512],
                   BR(f"cx{mi}{cc}{delta}"),
                   xT[:, PADC - 8 * delta:PADC - 8 * delta + CT],
                   start=(delta == 0), stop=False)
        for cc in range(2):
            mm(pxz1[:, cc * 512:(cc + 1) * 512], BR(f"cbrow{mi}{cc}"),
               BR("ones512"), False, True, kp=1)
        # silu: xc stays (t,q)
        xc = sp.tile([128, 1024], bf16, name="xc", tag="xc", bufs=2)
        nc.scalar.activation(xc[:], pxz1[:], AF.Silu)
        pxz2 = pp.tile([128, 1024], f32, name="pxz2", tag="A", bufs=1)
        for cc in range(2):
            mm(pxz2[:, cc * 512:(cc + 1) * 512],
               BR(f"z{mi}{cc}"), xT[:, PADC:PADC + CT], True, True)
        zg = sp.tile([128, 1024], bf16, name="zg", tag="zg", bufs=2)
        zv = zg[:].rearrange("p (h q t) -> p h q t", h=2, t=W)
        pzv = pxz2[:].rearrange("p (h t q) -> p h t q", h=2, q=8)
        nc.scalar.activation(zv, pzv.rearrange("p h t q -> p h q t"), AF.Silu)

        # x_proj -> [pB | pC]  (tag B, 2 banks)
        pbc = pp.tile([128, 1024], f32, name="pbc", tag="A", bufs=1)
        for cc in range(2):
            mm(pbc[:, 0:512], BR(f"wxB{mi}{cc}"),
               xc[:, cc * 512:(cc + 1) * 512], cc == 0, cc == 1)
            mm(pbc[:, 512:1024], BR(f"wxC{mi}{cc}"),
               xc[:, cc * 512:(cc + 1) * 512], cc == 0, cc == 1)
        Brep = sp.tile([128, CT], bf16, name="Brep", tag="Brep", bufs=2)
        nc.scalar.activation(Brep[:], pbc[:, 0:512], AF.Copy)
        Crep = sp.tile([128, CT], bf16, name="Crep", tag="Crep", bufs=2)
        crv = Crep[:].rearrange("p (q t) -> p q t", t=W)
        pcv = pbc[:, 512:1024].rearrange("p (t q) -> p t q", q=8)
        nc.scalar.activation(crv, pcv.rearrange("p t q -> p q t"), AF.Copy)

        # dt chain reusing tag B
        pd1 = pp.tile([128, 1024], f32, name="pd1", tag="A", bufs=1)
        for cc in range(2):
            mm(pd1[0:8, 0:512], BR(f"wxd{mi}{cc}"),
               xc[:, cc * 512:(cc + 1) * 512], cc == 0, cc == 1)
        dtin = sp.tile([8, CT], bf16, name="dtin", tag="dtin", bufs=2)
        nc.scalar.activation(dtin[:], pd1[0:8, 0:512], AF.Copy)
        pd2 = pp.tile([128, 1024], f32, name="pd2", tag="A", bufs=1)
        for cc in range(2):
            mm(pd2[:, cc * 512:(cc + 1) * 512], BR(f"dtw{mi}{cc}"),
               dtin[:], True, False, kp=8)
        for cc in range(2):
            mm(pd2[:, cc * 512:(cc + 1) * 512], BR(f"dtbrow{mi}{cc}"),
               BR("ones512"), False, True, kp=1)
        ez = sp.tile([128, 1024], bf16, name="ez", tag="ez", bufs=2)
        nc.scalar.activation(ez[:], pd2[:, 0:1024], AF.Exp)
        dtc = sp.tile([128, 1024], bf16, name="dtc", tag="dtc", bufs=2)
        nc.scalar.activation(dtc[:], ez[:], AF.Ln, bias=1.0)
        dtx = sp.tile([128, 1024], bf16, name="dtx", tag="dtx", bufs=2)
        nc.vector.tensor_mul(dtx[:], dtc[:], xc[:])
        # poison t=0 (first 8 cols of each cc half) AFTER dtx
        dpv = dtc[:].rearrange("p (h c) -> p h c", h=2)
        nc.vector.tensor_scalar(dpv[:, :, 0:8], dpv[:, :, 0:8], 0.0, 1.0e4,
                                OP.mult, OP.add)

        # ---- waves: per cc, 4 quads of 4 groups ----
        gated = sp.tile([128, 1024], bf16, name="gated", tag="gated", bufs=2)
        for cc in range(2):
            pY = pp.tile([128, CT], f32, name="pY", tag="py", bufs=1)
            for qd in range(4):
                dA = wavep.tile([128, 2048], bf16, name="dA", tag="dA", bufs=2)
                uu = wavep.tile([128, 2048], bf16, name="uu", tag="uu", bufs=1)
                for half in range(2):
                    g0 = qd * 4 + half * 2
                    for gi in range(2):
                        pP = pp.tile([128, 512], f32, name="pP", tag="P",
                                     bufs=2)
                        mm(pP[:], BR(f"dA{mi}{cc}{g0 + gi}"),
                           dtc[:, cc * 512:(cc + 1) * 512], True, True)
                        dav = dA[:, (half * 2 + gi) * 512:
                                 (half * 2 + gi + 1) * 512].rearrange(
                            "p (q t) -> p q t", t=W)
                        ppv = pP[:].rearrange("p (t q) -> p t q", q=8)
                        nc.scalar.activation(
                            dav, ppv.rearrange("p t q -> p q t"), AF.Exp)
                    for gi in range(2):
                        pX = pp.tile([128, 512], f32, name="pX", tag="X",
                                     bufs=2)
                        mm(pX[:], BR(f"delta{g0 + gi}"),
                           dtx[:, cc * 512:(cc + 1) * 512], True, True)
                        uv = uu[:, (half * 2 + gi) * 512:
                                (half * 2 + gi + 1) * 512].rearrange(
                            "p (q t) -> p q t", t=W)
                        pxv = pX[:].rearrange("p (t q) -> p t q", q=8)
                        brv = Brep[:].rearrange("p (t q) -> p t q", q=8)
                        nc.vector.tensor_mul(
                            uv, pxv.rearrange("p t q -> p q t"),
                            brv.rearrange("p t q -> p q t"))
                hh = wavep.tile([128, 2048], bf16, name="hh", tag="hh", bufs=2)
                nc.vector.tensor_tensor_scan(hh[:], dA[:], uu[:], 0.0,
                                             OP.mult, OP.add)
                yh = wavep.tile([128, 2048], bf16, name="yh", tag="yh", bufs=2)
                crq = Crep[:].rearrange("p (x c) -> p x c", x=1) \
                    .broadcast_to([128, 4, 512])
                nc.gpsimd.tensor_mul(
                    yh[:].rearrange("p (r c) -> p r c", r=4),
                    hh[:].rearrange("p (r c) -> p r c", r=4), crq)
                for gi in range(4):
                    mm(pY[:], BR(f"sum{qd * 4 + gi}"),
                       yh[:, gi * 512:(gi + 1) * 512],
                       qd == 0 and gi == 0, qd == 3 and gi == 3)
            yg = sp.tile([128, CT], bf16, name="yg", tag="yg", bufs=2)
            xcv = xc[:, cc * 512:(cc + 1) * 512].rearrange(
                "p (t q) -> p t q", q=8)
            nc.vector.scalar_tensor_tensor(
                yg[:], xcv.rearrange("p t q -> p q t"), VP(f"Dp{mi}{cc}"),
                pY[:], OP.mult, OP.add)
            nc.vector.tensor_mul(gated[:, cc * 512:(cc + 1) * 512], yg[:],
                                 zg[:, cc * 512:(cc + 1) * 512])
        po = pp.tile([128, CT], f32, name="po", tag="py", bufs=1)
        for cc in range(2):
            mm(po[:], BR(f"wout{mi}{cc}"), gated[:, cc * 512:(cc + 1) * 512],
               cc == 0, cc == 1)
        if bname in ("t0", "t1"):
            res = sp.tile([128, CT], bf16, name="res", tag="res", bufs=2)
            xtv = xT[:, PADC:PADC + CT].rearrange("p (t q) -> p t q", q=8)
            nc.vector.tensor_add(res[:], po[:],
                                 xtv.rearrange("p t q -> p q t"))
            nc.scalar.dma_start(aps[f"scr_s_{bname}"][:, it * CT:(it + 1) * CT],
                                res[:])
            tile_stats(res[:], statA[bname], it, sp, pp, ("pst", 1), ("py", 1))
        else:
            ym = sp.tile([128, CT], bf16, name="ym", tag="res", bufs=2)
            nc.scalar.activation(ym[:], po[:], AF.Copy)
            nc.scalar.dma_start(aps[f"scr_ym_{bname}"][:, it * CT:(it + 1) * CT],
                                ym[:])

    # ---------------- mix phase (one c0 tile) ------------------------
    def mix_tile(it, sp, pp):
        cat = {}
        for bname in ("n", "l"):
            t_ = sp.tile([128, CT], bf16, name=f"ym{bname}", tag=f"ym{bname}",
                         bufs=2)
            nc.sync.dma_start(t_[:],
                              aps[f"scr_ym_{bname}"][:, it * CT:(it + 1) * CT])
            cat[bname] = t_
        for mc, bname in enumerate(("n", "l")):
            bi = 0 if bname == "n" else 3
            pmxb = pp.tile([128, 1024], f32, name="pmxb", tag="A", bufs=1)
            pmx = pmxb[:, mc * 512:(mc + 1) * 512]
            for kc, bn2 in enumerate(("n", "l")):
                mm(pmx, BR(f"mix{kc}{mc}"), cat[bn2][:], kc == 0, kc == 1)
            ms = sp.tile([128, CT], bf16, name="ms", tag="ms", bufs=2)
            nc.scalar.activation(ms[:], pmx, AF.Silu, bias=VP(f"mixb{mc}"))
            t2 = sp.tile([128, CT], bf16, name="t2m", tag="t2m", bufs=2)
            nc.vector.tensor_add(t2[:], cat[bname][:], ms[:])
            xtile = sp.tile([128, CT], bf16, name="xre", tag="xre", bufs=2)
            qb = (0 if bname == "n" else 1) * CBLK
            nc.sync.dma_start(xtile[:],
                              aps["xqt"][:, qb + it * CT:qb + (it + 1) * CT])
            res = sp.tile([128, CT], bf16, name="resm", tag="resm", bufs=2)
            nc.vector.tensor_add(res[:], t2[:], xtile[:])
            nc.scalar.dma_start(aps[f"scr_s_{bname}"][:, it * CT:(it + 1) * CT],
                                res[:])
            tile_stats(res[:], statA[bname], it, sp, pp, ("pst", 1), ("py", 1))

    # ---------------- phase C: LN -> FFN -> LN -> out ----------------
    def phase_c_block(bname, mi, bi, sp, pp):
        rA, nmA = ln_finish(statA[bname], sp, f"A{bname}")

        for it in range(TPB):
            rr, rnm = load_rst(rA, nmA, it, sp)
            sld = sp.tile([128, CT], bf16, name="sld", tag="sld", bufs=2)
            nc.sync.dma_start(sld[:],
                              aps[f"scr_s_{bname}"][:, it * CT:(it + 1) * CT])
            prep = pp.tile([128, CT], f32, name="prep", tag="prep", bufs=2)
            mm(prep[:], BR("onescol"), rr[0:1, :], True, True, kp=1)
            t1 = sp.tile([128, CT], bf16, name="t1", tag="t1", bufs=2)
            nc.vector.tensor_mul(t1[:], sld[:], prep[:])
            hh4 = sp.tile([128, 2048], bf16, name="hh4", tag="hh4", bufs=2)
            for cp in range(2):
                pf = pp.tile([128, 1024], f32, name="pf", tag="pf", bufs=2)
                for ci in range(2):
                    c4 = cp * 2 + ci
                    mm(pf[:, ci * 512:(ci + 1) * 512],
                       BR(f"w1g{mi}")[:, c4 * 128:(c4 + 1) * 128], t1[:],
                       True, False)
                    mm(pf[:, ci * 512:(ci + 1) * 512], BR(f"gb1row{mi}{c4}"),
                       rnm[0:2, :], False, True, kp=2)
                nc.scalar.activation(hh4[:, cp * 1024:(cp + 1) * 1024], pf[:],
                                     AF.Prelu, alpha=0.01)
            pf2 = pp.tile([128, CT], f32, name="pf2", tag="pf2", bufs=2)
            for c4 in range(4):
                mm(pf2[:], BR(f"ff2{mi}{c4}"),
                   hh4[:, c4 * 512:(c4 + 1) * 512], c4 == 0, False)
            mm(pf2[:], BR(f"af2row{mi}"), rnm[0:2, :], False, True, kp=2)
            sft = sp.tile([128, CT], bf16, name="sft", tag="sft", bufs=2)
            nc.vector.scalar_tensor_tensor(sft[:], t1[:], VP(f"ang{mi}"),
                                           pf2[:], OP.mult, OP.add)
            nc.scalar.dma_start(
                aps["scr_sf"][:, bi * CBLK + it * CT:bi * CBLK + (it + 1) * CT],
                sft[:])
            tile_stats(sft[:], statF[bname], it, sp, pp, ("prep", 2), ("pf2", 2))
        rF, nmF = ln_finish(statF[bname], sp, f"F{bname}")
        for it in range(TPB):
            rrF, rnmF = load_rst(rF, nmF, it, sp)
            prepF = pp.tile([128, CT], f32, name="prepF", tag="prep", bufs=2)
            mm(prepF[:], BR("onescol"), rrF[0:1, :], True, True, kp=1)
            sfl = sp.tile([128, CT], bf16, name="sfl", tag="sfl", bufs=2)
            nc.sync.dma_start(
                sfl[:],
                aps["scr_sf"][:, bi * CBLK + it * CT:bi * CBLK + (it + 1) * CT])
            t2 = sp.tile([128, CT], bf16, name="t2c", tag="t2c", bufs=2)
            nc.vector.tensor_mul(t2[:], sfl[:], prepF[:])
            pr2 = pp.tile([128, CT], f32, name="pr2", tag="pf2", bufs=2)
            mm(pr2[:], BR(f"flrow{mi}"), rnmF[0:2, :], True, True, kp=2)
            n2 = sp.tile([128, CT], f32, name="n2", tag="n2", bufs=2)
            nc.vector.scalar_tensor_tensor(n2[:], t2[:], VP(f"flg{mi}"),
                                           pr2[:], OP.mult, OP.add)
            nc.scalar.dma_start(
                aps["outp"][:, bi * CBLK + it * CT:bi * CBLK + (it + 1) * CT],
                n2[:])

    # ------------------------- schedule ------------------------------
    with tc.tile_pool(name="a_sb", bufs=1) as sp, \
            tc.tile_pool(name="a_wv", bufs=1) as wavep:
        with tc.tile_pool(name="a_ps", bufs=1, space="PSUM") as pp:
            for bname, mi, bi in [("n", 0, 0), ("l", 2, 3)]:
                for it in range(TPB):
                    phase_a_tile(bname, mi, bi, it, sp, pp, wavep)
            for it in range(TPB):
                mix_tile(it, sp, pp)
            for bname, mi, bi in [("t0", 1, 1), ("t1", 1, 2)]:
                for it in range(TPB):
                    phase_a_tile(bname, mi, bi, it, sp, pp, wavep)
        with tc.tile_pool(name="c_ps", bufs=1, space="PSUM") as pp2:
            for bname, mi, bi in BLOCKS:
                phase_c_block(bname, mi, bi, sp, pp2)


def _build_program(wp, vp, bp):
    nc = bacc.Bacc("TRN2", target_bir_lowering=False, debug=False,
                   num_devices=N_CORES)
    aps = {}
    aps["xpack"] = nc.dram_tensor("xpack", [128, 4 * TPB * TW], bf16,
                                  kind="ExternalInput").ap()
    aps["xqt"] = nc.dram_tensor("xqt", [128, 2 * CBLK], bf16,
                                kind="ExternalInput").ap()
    aps["vpack"] = nc.dram_tensor("vpack", [128, vp.n], f32,
                                  kind="ExternalInput").ap()
    aps["bpack"] = nc.dram_tensor("bpack", [128, bp.n], bf16,
                                  kind="ExternalInput").ap()
    aps["outp"] = nc.dram_tensor("outp", [128, 4 * CBLK], f32,
                                 kind="ExternalOutput").ap()
    for bname, _, _ in BLOCKS:
        aps[f"scr_s_{bname}"] = nc.dram_tensor(
            f"scr_s_{bname}", [128, CBLK], bf16).ap()
    for bname in ("n", "l"):
        aps[f"scr_ym_{bname}"] = nc.dram_tensor(
            f"scr_ym_{bname}", [128, CBLK], bf16).ap()
    aps["scr_sf"] = nc.dram_tensor("scr_sf", [128, 4 * CBLK], bf16).ap()

    with tile.TileContext(nc) as tc:
        with ExitStack() as ctx:
            _emit(ctx, tc, nc, aps, wp, vp, bp)
    nc.compile()
    return nc


_CACHE = {}


def kernel(**inputs):
    inputs = {k: np.asarray(v) for k, v in inputs.items()}
    wp, vp, bp = _host_pack(inputs)
    if "prog" not in _CACHE:
        _CACHE["prog"] = _build_program(wp, vp, bp)
    nc = _CACHE["prog"]
    vpack, bpack = vp.build(), bp.build()
    in_maps = []
    for b in range(B):
        xp = np.concatenate([
            _pack_x(inputs["x_node"][b]),
            _pack_x(inputs["x_trace"][b][:, 0:128]),
            _pack_x(inputs["x_trace"][b][:, 128:256]),
            _pack_x(inputs["x_log"][b]),
        ], axis=1)
        xqt = np.concatenate([_pack_x_qt(inputs["x_node"][b]),
                              _pack_x_qt(inputs["x_log"][b])], axis=1)
        in_maps.append({"xpack": np.ascontiguousarray(xp), "xqt": xqt,
                        "vpack": vpack, "bpack": bpack})
    res = run_bass_kernel_spmd(nc, in_maps, list(range(N_CORES)))
    out = np.empty((B, W, 2 * N + E, D), np.float32)
    for b in range(B):
        op = res.results[b]["outp"]                     # (128, 4*CBLK)
        for bi, j0 in [(0, 0), (1, N), (2, N + 128), (3, N + E)]:
            blk = op[:, bi * CBLK:(bi + 1) * CBLK]
            arr = blk.reshape(D, TPB, 8, W)             # (D, it, q, t)
            out[b, :, j0:j0 + 128, :] = arr.transpose(3, 1, 2, 0) \
                .reshape(W, 128, D)
    return out


# revision 21
# speedup vs baseline: 1.6543x; 1.0201x over previous
"""Trainium2 Bass kernel for nn_Encoder (tri-modal Mamba encoder), v2.

kernel(**inputs) takes FULL unsharded numpy inputs and returns the FULL
output (B, W, 2N+E, D). Batch B=8 is sharded across 8 NeuronCores (pure
data parallel); params are replicated.

v2 design (per core, one batch element):
- Host pre-packs activations into bf16 feature-major tiles with column
  order (t, q) and 24 zero pad columns per 8-seq tile, so the causal
  depthwise conv becomes 4 time-shifted accumulating PE matmuls with
  plain contiguous operands (shift by d = 8*d columns).
- The (q, t) reorder the scan needs happens inside ACT/DVE ops via
  3D/4D strided output APs (dA exp, u-mult, Crep copy, z silu).
- A[d,s] folds into the dt-expansion weights (deltaA, bf16) so the dA
  exp needs no per-group scale and pairs of groups share 1024-wide ACT
  ops. conv_b/dt_b biases are K=2 rank-2 matmuls so silu/softplus pair
  across the two DI halves.
- LayerNorm affine folds into host-premultiplied weights plus K<=2
  broadcast matmuls; rsqrt = exp(-0.5*ln(var+eps)); Prelu for LeakyReLU;
  only silu blocks switch ACT tables.
- Engines: PE expansions+sums+conv+proj, ACT exps/silus/evacuations,
  DVE u-mult + 2048-wide scans, Pool yh-mults + squares.
- Weights stage via gpsimd cast-DMA (f32 DRAM -> f32r SBUF).
"""

import ml_dtypes
import numpy as np
from contextlib import ExitStack

import concourse.bass as bass
import concourse.tile as tile
from concourse import bacc, mybir
from concourse.bass_utils import run_bass_kernel_spmd

D, DI, SS, KK, RR = 128, 256, 16, 4, 8
B, W, N, E = 8, 64, 128, 256
Q = 128                      # seqs per block
CT = 512                     # columns (tokens) per tile = 8 seqs
TPB = Q * W // CT            # 16 tiles per block
CBLK = Q * W                 # 8192 tokens per block
PADC = 24                    # zero pad columns per tile (3 t-steps * 8 seqs)
TW = PADC + CT               # packed tile width
f32 = mybir.dt.float32
f32r = mybir.dt.float32r
bf16 = mybir.dt.bfloat16
AF = mybir.ActivationFunctionType
OP = mybir.AluOpType
N_CORES = 8
LN_EPS = 1e-5
NW = CBLK // 128             # 64 token-chunks per stat partition

# blocks: (name, modality index, block index in xpack/outp)
BLOCKS = [("n", 0, 0), ("t0", 1, 1), ("t1", 1, 2), ("l", 2, 3)]


class Pack:
    """Column allocator for a (128, *) packed parameter array."""

    def __init__(self, np_dtype):
        self.cols = []
        self.off = {}
        self.n = 0
        self.np_dtype = np_dtype

    def add(self, name, arr):
        arr = np.asarray(arr, np.float64)
        assert arr.ndim == 2 and arr.shape[0] <= 128
        a = np.zeros((128, arr.shape[1]), np.float64)
        a[: arr.shape[0]] = arr
        self.off[name] = (self.n, arr.shape[1])
        self.cols.append(a)
        self.n += arr.shape[1]

    def build(self):
        return np.concatenate(self.cols, axis=1).astype(self.np_dtype)


def _host_pack(inp):
    wp = Pack(np.float32)      # -> f32r on device via gpsimd cast DMA
    bp = Pack(ml_dtypes.bfloat16)
    vp = Pack(np.float32)      # per-partition vectors

    # --- bf16 patterns: sum, delta, deltaA ---
    for g in range(16):
        sm = np.zeros((128, 128))
        for k in range(128):
            sm[k, g * 8 + k // 16] = 1.0
        bp.add(f"sum{g}", sm)
        dl = np.zeros((128, 128))
        for j in range(128):
            dl[g * 8 + j // 16, j] = 1.0
        bp.add(f"delta{g}", dl)
    for m in range(3):
        A = -np.exp(np.asarray(inp["mp_Alog"][m], np.float64))    # (DI, S)
        for cc in range(2):
            for g in range(16):
                dlA = np.zeros((128, 128))
                for j in range(128):
                    dlA[g * 8 + j // 16, j] = A[cc * 128 + g * 8 + j // 16,
                                                j % 16]
                bp.add(f"dA{m}{cc}{g}", dlA)

    # --- f32r weights ---
    for m in range(3):
        win = np.asarray(inp["mp_in"][m], np.float64)              # (D, 2DI)
        cw = np.asarray(inp["mp_conv_w"][m], np.float64)           # (DI, K)
        for cc in range(2):
            sl = slice(cc * 128, (cc + 1) * 128)
            for delta in range(4):                                 # t shift
                k = 3 - delta
                bp.add(f"cx{m}{cc}{delta}", win[:, sl] * cw[sl, k][None, :])
            bp.add(f"z{m}{cc}", win[:, 256 + cc * 128:256 + (cc + 1) * 128])
        wxp = np.asarray(inp["mp_xproj"][m], np.float64)           # (DI, R+2S)
        for cc in range(2):
            blk = wxp[cc * 128:(cc + 1) * 128]
            bp.add(f"wxB{m}{cc}", np.tile(blk[:, RR:RR + SS], (1, 8)))
            bp.add(f"wxC{m}{cc}", np.tile(blk[:, RR + SS:], (1, 8)))
            bp.add(f"wxd{m}{cc}", blk[:, :RR])                     # (128, 8)
        dtw = np.asarray(inp["mp_dt_w"][m], np.float64)            # (R, DI)
        for cc in range(2):
            bp.add(f"dtw{m}{cc}", dtw[:, cc * 128:(cc + 1) * 128])
        wout = np.asarray(inp["mp_out"][m], np.float64)            # (DI, D)
        for cc in range(2):
            bp.add(f"wout{m}{cc}", wout[cc * 128:(cc + 1) * 128])
        ang = np.asarray(inp["an_g"][m], np.float64)
        anb = np.asarray(inp["an_b"][m], np.float64)
        ff1 = np.asarray(inp["ff1_w"][m], np.float64)              # (D, 4D)
        bp.add(f"w1g{m}", ang[:, None] * ff1)                      # LN-A fold
        ff2 = np.asarray(inp["ff2_w"][m], np.float64)              # (4D, D)
        for c4 in range(4):
            bp.add(f"ff2{m}{c4}", ff2[c4 * 128:(c4 + 1) * 128])
        # rank-2 rows (2 partitions used). nm rows carry MINUS sign since
        # the runtime nm tile holds +mean*rstd.
        for cc in range(2):
            bp.add(f"cbrow{m}{cc}",
                   np.asarray(inp["mp_conv_b"][m], np.float64)
                   [None, cc * 128:(cc + 1) * 128])
            bp.add(f"dtbrow{m}{cc}",
                   np.asarray(inp["mp_dt_b"][m], np.float64)
                   [None, cc * 128:(cc + 1) * 128])
        gam1 = ang @ ff1                                           # (4D,)
        bet1 = anb @ ff1 + np.asarray(inp["ff1_b"][m], np.float64)
        for c4 in range(4):
            bp.add(f"gb1row{m}{c4}",
                   np.stack([bet1[c4 * 128:(c4 + 1) * 128],
                             -gam1[c4 * 128:(c4 + 1) * 128]]))
        bp.add(f"af2row{m}", np.stack([np.asarray(inp["ff2_b"][m],
                                                  np.float64) + anb, -ang]))
        bp.add(f"flrow{m}",
               np.stack([np.asarray(inp["fln_b"][m], np.float64),
                         -np.asarray(inp["fln_g"][m], np.float64)]))
    mixw = np.asarray(inp["mix_w"], np.float64)
    for kc in range(2):
        for mc in range(2):
            bp.add(f"mix{kc}{mc}", mixw[kc * 128:(kc + 1) * 128,
                                        mc * 128:(mc + 1) * 128])
    bp.add("ones512", np.ones((1, 512)))
    bp.add("onesD", np.full((128, 1), 1.0 / D))
    bp.add("onescol", np.ones((1, 128)))
    oc32 = np.zeros((33, 128))
    oc32[32, :] = 1.0
    bp.add("onescol32", oc32)

    vp.add("eps", np.full((128, 1), LN_EPS))
    for m in range(3):
        for cc in range(2):
            vp.add(f"Dp{m}{cc}", np.asarray(inp["mp_D"][m], np.float64)
                   [cc * 128:(cc + 1) * 128, None])
        vp.add(f"ang{m}", np.asarray(inp["an_g"][m], np.float64)[:, None])
        vp.add(f"flg{m}", np.asarray(inp["fln_g"][m], np.float64)[:, None])
    for mc in range(2):
        vp.add(f"mixb{mc}", np.asarray(inp["mix_b"], np.float64)
               [mc * 128:(mc + 1) * 128, None])
    return wp, vp, bp


def _pack_x_qt(xb):
    """(W, Qs, D) f32 -> (128, Qs*W) bf16, (q,t)-major contiguous."""
    Wl, Qs, Dl = xb.shape
    return np.ascontiguousarray(
        xb.transpose(2, 1, 0).reshape(Dl, Qs * Wl)).astype(ml_dtypes.bfloat16)


def _pack_x(xb):
    """(W, Qs, D) f32 -> (128, ntile*TW) bf16 padded (t,q)-major tiles."""
    Wl, Qs, Dl = xb.shape
    ntile = Qs // 8
    out = np.zeros((128, ntile * TW), np.float32)
    for i in range(ntile):
        sl = xb[:, i * 8:(i + 1) * 8, :]           # (W, 8, D)
        out[:, i * TW + PADC:(i + 1) * TW] = \
            sl.transpose(2, 0, 1).reshape(Dl, W * 8)
    return out.astype(ml_dtypes.bfloat16)


def _emit(ctx, tc, nc, aps, wp, vp, bp):
    wpool = ctx.enter_context(tc.tile_pool(name="weights", bufs=1))
    bw = wpool.tile([128, bp.n], bf16, name="bw")
    nc.sync.dma_start(bw[:], aps["bpack"][:])
    vec = wpool.tile([128, vp.n], f32, name="vec")
    nc.sync.dma_start(vec[:], aps["vpack"][:])
    rAll = wpool.tile([33, CBLK], bf16, name="rAll")
    nc.vector.memset(rAll[0:1, :], 1.0)

    def BR(name):
        o, c = bp.off[name]
        return bw[:, o:o + c]

    def VP(name):
        o, c = vp.off[name]
        return vec[:, o:o + c]

    def mm(psum_ap, lhsT_ap, rhs_ap, start, stop, kp=None):
        if kp is not None:
            lhsT_ap = lhsT_ap[:kp, :]
            if rhs_ap.partition_size() != kp:
                rhs_ap = rhs_ap[:kp, :]
        nc.tensor.matmul(psum_ap, lhsT_ap, rhs_ap, start=start, stop=stop)

    statA, statF = {}, {}
    for bname, _, _ in BLOCKS:
        statA[bname] = wpool.tile([128, 2 * NW], f32, name=f"sA_{bname}")
        statF[bname] = wpool.tile([128, 2 * NW], f32, name=f"sF_{bname}")

    def tile_stats(res_ap, statT, it, sp, pp, tagm, tagq):
        """res (128,512) bf16 SBUF -> mean/meansq sums -> statT chunk it."""
        s2 = sp.tile([128, CT], bf16, name="s2", tag="s2", bufs=2)
        nc.gpsimd.tensor_mul(s2[:], res_ap, res_ap)
        pm = pp.tile([128, CT], f32, name="pm", tag=tagm[0], bufs=tagm[1])
        mm(pm[0:1, :], BR("onesD"), res_ap, True, True)
        pq = pp.tile([128, CT], f32, name="pq", tag=tagq[0], bufs=tagq[1])
        mm(pq[0:1, :], BR("onesD"), s2[:], True, True)
        stv = sp.tile([1, 1024], f32, name="stv", tag="stv", bufs=2)
        nc.scalar.activation(stv[:, 0:512], pm[0:1, :], AF.Copy)
        nc.scalar.activation(stv[:, 512:1024], pq[0:1, :], AF.Copy)
        p0 = it * 8
        nc.scalar.dma_start(statT[p0:p0 + 8, 0:NW],
                            stv[:, 0:512].rearrange("x (p w) -> x p w", w=NW))
        nc.scalar.dma_start(statT[p0:p0 + 8, NW:2 * NW],
                            stv[:, 512:1024].rearrange("x (p w) -> x p w",
                                                       w=NW))

    def ln_finish(statT, sp, tag):
        """statT -> bf16 (128, NW) tiles r (rstd) and nm (mean*rstd)."""
        m_t, q_t = statT[:, 0:NW], statT[:, NW:2 * NW]
        var = sp.tile([128, NW], f32, name=f"var{tag}", tag="lnv", bufs=2)
        nc.vector.tensor_mul(var[:], m_t, m_t)
        nc.vector.tensor_sub(var[:], q_t, var[:])
        lg = sp.tile([128, NW], f32, name=f"lg{tag}", tag="lnl", bufs=2)
        nc.scalar.activation(lg[:], var[:], AF.Ln, bias=VP("eps"))
        r_t = sp.tile([128, NW], bf16, name=f"r{tag}", tag=f"lnr{tag}")
        nc.scalar.activation(r_t[:], lg[:], AF.Exp, scale=-0.5)
        nm = sp.tile([128, NW], bf16, name=f"nm{tag}", tag=f"lnm{tag}")
        nc.vector.tensor_mul(nm[:], m_t, r_t[:])
        return r_t, nm

    def load_rst_block(r_t, nm):
        """Gather the whole block's r/nm chunk-major tiles into row tiles."""
        nc.sync.dma_start(rAll[32:33, :].rearrange("x (p w) -> x p w", w=NW),
                          r_t[:, :])
        nc.sync.dma_start(rAll[1:2, :].rearrange("x (p w) -> x p w", w=NW),
                          nm[:, :])

    # ---------------- phase A: mamba for one tile --------------------
    def phase_a_tile(bname, mi, bi, it, sp, pp, wavep):
        base = (bi * TPB + it) * TW
        xT = sp.tile([128, TW], bf16, name="xT", tag="xT", bufs=2)
        nc.sync.dma_start(xT[:], aps["xpack"][:, base:base + TW])

        # in_proj + conv: pxz (2 banks) used twice: xc pair, then z pair
        pxz1 = pp.tile([128, 1024], f32, name="pxz1", tag="A", bufs=1)
        for cc in range(2):
            for delta in range(4):
                mm(pxz1[:, cc * 512:(cc + 1) * 512],
                   BR(f"cx{mi}{cc}{delta}"),
                   xT[:, PADC - 8 * delta:PADC - 8 * delta + CT],
                   start=(delta == 0), stop=False)
        for cc in range(2):
            mm(pxz1[:, cc * 512:(cc + 1) * 512], BR(f"cbrow{mi}{cc}"),
               BR("ones512"), False, True, kp=1)
        # silu: xc stays (t,q)
        xc = sp.tile([128, 1024], bf16, name="xc", tag="xc", bufs=2)
        nc.scalar.activation(xc[:], pxz1[:], AF.Silu)
        pxz2 = pp.tile([128, 1024], f32, name="pxz2", tag="A", bufs=1)
        for cc in range(2):
            mm(pxz2[:, cc * 512:(cc + 1) * 512],
               BR(f"z{mi}{cc}"), xT[:, PADC:PADC + CT], True, True)
        zg = sp.tile([128, 1024], bf16, name="zg", tag="zg", bufs=2)
        zv = zg[:].rearrange("p (h q t) -> p h q t", h=2, t=W)
        pzv = pxz2[:].rearrange("p (h t q) -> p h t q", h=2, q=8)
        nc.scalar.activation(zv, pzv.rearrange("p h t q -> p h q t"), AF.Silu)

        # x_proj -> [pB | pC]  (tag B, 2 banks)
        pbc = pp.tile([128, 1024], f32, name="pbc", tag="A", bufs=1)
        for cc in range(2):
            mm(pbc[:, 0:512], BR(f"wxB{mi}{cc}"),
               xc[:, cc * 512:(cc + 1) * 512], cc == 0, cc == 1)
            mm(pbc[:, 512:1024], BR(f"wxC{mi}{cc}"),
               xc[:, cc * 512:(cc + 1) * 512], cc == 0, cc == 1)
        Brep = sp.tile([128, CT], bf16, name="Brep", tag="Brep", bufs=2)
        nc.scalar.activation(Brep[:], pbc[:, 0:512], AF.Copy)
        Crep = sp.tile([128, CT], bf16, name="Crep", tag="Crep", bufs=2)
        crv = Crep[:].rearrange("p (q t) -> p q t", t=W)
        pcv = pbc[:, 512:1024].rearrange("p (t q) -> p t q", q=8)
        nc.scalar.activation(crv, pcv.rearrange("p t q -> p q t"), AF.Copy)

        # dt chain reusing tag B
        pd1 = pp.tile([128, 1024], f32, name="pd1", tag="A", bufs=1)
        for cc in range(2):
            mm(pd1[0:8, 0:512], BR(f"wxd{mi}{cc}"),
               xc[:, cc * 512:(cc + 1) * 512], cc == 0, cc == 1)
        dtin = sp.tile([8, CT], bf16, name="dtin", tag="dtin", bufs=2)
        nc.scalar.activation(dtin[:], pd1[0:8, 0:512], AF.Copy)
        pd2 = pp.tile([128, 1024], f32, name="pd2", tag="A", bufs=1)
        for cc in range(2):
            mm(pd2[:, cc * 512:(cc + 1) * 512], BR(f"dtw{mi}{cc}"),
               dtin[:], True, False, kp=8)
        for cc in range(2):
            mm(pd2[:, cc * 512:(cc + 1) * 512], BR(f"dtbrow{mi}{cc}"),
               BR("ones512"), False, True, kp=1)
        ez = sp.tile([128, 1024], bf16, name="ez", tag="ez", bufs=2)
        nc.scalar.activation(ez[:], pd2[:, 0:1024], AF.Exp)
        dtc = sp.tile([128, 1024], bf16, name="dtc", tag="dtc", bufs=2)
        nc.scalar.activation(dtc[:], ez[:], AF.Ln, bias=1.0)
        dtx = sp.tile([128, 1024], bf16, name="dtx", tag="dtx", bufs=2)
        nc.vector.tensor_mul(dtx[:], dtc[:], xc[:])
        # poison t=0 (first 8 cols of each cc half) AFTER dtx
        dpv = dtc[:].rearrange("p (h c) -> p h c", h=2)
        nc.vector.tensor_scalar(dpv[:, :, 0:8], dpv[:, :, 0:8], 0.0, 1.0e4,
                                OP.mult, OP.add)

        # ---- waves: per cc, 4 quads of 4 groups ----
        gated = sp.tile([128, 1024], bf16, name="gated", tag="gated", bufs=2)
        for cc in range(2):
            pY = pp.tile([128, CT], f32, name="pY", tag="py", bufs=1)
            for qd in range(4):
                dA = wavep.tile([128, 2048], bf16, name="dA", tag="dA", bufs=2)
                uu = wavep.tile([128, 2048], bf16, name="uu", tag="uu", bufs=1)
                for half in range(2):
                    g0 = qd * 4 + half * 2
                    for gi in range(2):
                        pP = pp.tile([128, 512], f32, name="pP", tag="P",
                                     bufs=2)
                        mm(pP[:], BR(f"dA{mi}{cc}{g0 + gi}"),
                           dtc[:, cc * 512:(cc + 1) * 512], True, True)
                        dav = dA[:, (half * 2 + gi) * 512:
                                 (half * 2 + gi + 1) * 512].rearrange(
                            "p (q t) -> p q t", t=W)
                        ppv = pP[:].rearrange("p (t q) -> p t q", q=8)
                        nc.scalar.activation(
                            dav, ppv.rearrange("p t q -> p q t"), AF.Exp)
                    for gi in range(2):
                        pX = pp.tile([128, 512], f32, name="pX", tag="X",
                                     bufs=2)
                        mm(pX[:], BR(f"delta{g0 + gi}"),
                           dtx[:, cc * 512:(cc + 1) * 512], True, True)
                        uv = uu[:, (half * 2 + gi) * 512:
                                (half * 2 + gi + 1) * 512].rearrange(
                            "p (q t) -> p q t", t=W)
                        pxv = pX[:].rearrange("p (t q) -> p t q", q=8)
                        brv = Brep[:].rearrange("p (t q) -> p t q", q=8)
                        nc.vector.tensor_mul(
                            uv, pxv.rearrange("p t q -> p q t"),
                            brv.rearrange("p t q -> p q t"))
                hh = wavep.tile([128, 2048], bf16, name="hh", tag="hh", bufs=2)
                nc.vector.tensor_tensor_scan(hh[:], dA[:], uu[:], 0.0,
                                             OP.mult, OP.add)
                yh = wavep.tile([128, 2048], bf16, name="yh", tag="yh", bufs=2)
                crq = Crep[:].rearrange("p (x c) -> p x c", x=1) \
                    .broadcast_to([128, 4, 512])
                nc.gpsimd.tensor_mul(
                    yh[:].rearrange("p (r c) -> p r c", r=4),
                    hh[:].rearrange("p (r c) -> p r c", r=4), crq)
                for gi in range(4):
                    mm(pY[:], BR(f"sum{qd * 4 + gi}"),
                       yh[:, gi * 512:(gi + 1) * 512],
                       qd == 0 and gi == 0, qd == 3 and gi == 3)
            yg = sp.tile([128, CT], bf16, name="yg", tag="yg", bufs=2)
            xcv = xc[:, cc * 512:(cc + 1) * 512].rearrange(
                "p (t q) -> p t q", q=8)
            nc.vector.scalar_tensor_tensor(
                yg[:], xcv.rearrange("p t q -> p q t"), VP(f"Dp{mi}{cc}"),
                pY[:], OP.mult, OP.add)
            nc.vector.tensor_mul(gated[:, cc * 512:(cc + 1) * 512], yg[:],
                                 zg[:, cc * 512:(cc + 1) * 512])
        po = pp.tile([128, CT], f32, name="po", tag="py", bufs=1)
        for cc in range(2):
            mm(po[:], BR(f"wout{mi}{cc}"), gated[:, cc * 512:(cc + 1) * 512],
               cc == 0, cc == 1)
        if bname in ("t0", "t1"):
            res = sp.tile([128, CT], bf16, name="res", tag="res", bufs=2)
            xtv = xT[:, PADC:PADC + CT].rearrange("p (t q) -> p t q", q=8)
            nc.vector.tensor_add(res[:], po[:],
                                 xtv.rearrange("p t q -> p q t"))
            nc.scalar.dma_start(aps[f"scr_s_{bname}"][:, it * CT:(it + 1) * CT],
                                res[:])
            tile_stats(res[:], statA[bname], it, sp, pp, ("pst", 1), ("py", 1))
        else:
            ym = sp.tile([128, CT], bf16, name="ym", tag="res", bufs=2)
            nc.scalar.activation(ym[:], po[:], AF.Copy)
            nc.scalar.dma_start(aps[f"scr_ym_{bname}"][:, it * CT:(it + 1) * CT],
                                ym[:])

    # ---------------- mix phase (one c0 tile) ------------------------
    def mix_tile(it, sp, pp):
        cat = {}
        for bname in ("n", "l"):
            t_ = sp.tile([128, CT], bf16, name=f"ym{bname}", tag=f"ym{bname}",
                         bufs=2)
            nc.sync.dma_start(t_[:],
                              aps[f"scr_ym_{bname}"][:, it * CT:(it + 1) * CT])
            cat[bname] = t_
        for mc, bname in enumerate(("n", "l")):
            bi = 0 if bname == "n" else 3
            pmxb = pp.tile([128, 1024], f32, name="pmxb", tag="A", bufs=1)
            pmx = pmxb[:, mc * 512:(mc + 1) * 512]
            for kc, bn2 in enumerate(("n", "l")):
                mm(pmx, BR(f"mix{kc}{mc}"), cat[bn2][:], kc == 0, kc == 1)
            ms = sp.tile([128, CT], bf16, name="ms", tag="ms", bufs=2)
            nc.scalar.activation(ms[:], pmx, AF.Silu, bias=VP(f"mixb{mc}"))
            t2 = sp.tile([128, CT], bf16, name="t2m", tag="t2m", bufs=2)
            nc.vector.tensor_add(t2[:], cat[bname][:], ms[:])
            xtile = sp.tile([128, CT], bf16, name="xre", tag="xre", bufs=2)
            qb = (0 if bname == "n" else 1) * CBLK
            nc.sync.dma_start(xtile[:],
                              aps["xqt"][:, qb + it * CT:qb + (it + 1) * CT])
            res = sp.tile([128, CT], bf16, name="resm", tag="resm", bufs=2)
            nc.vector.tensor_add(res[:], t2[:], xtile[:])
            nc.scalar.dma_start(aps[f"scr_s_{bname}"][:, it * CT:(it + 1) * CT],
                                res[:])
            tile_stats(res[:], statA[bname], it, sp, pp, ("pst", 1), ("py", 1))

    # ---------------- phase C: LN -> FFN -> LN -> out ----------------
    def phase_c_block(bname, mi, bi, sp, pp):
        rA, nmA = ln_finish(statA[bname], sp, f"A{bname}")
        load_rst_block(rA, nmA)

        for it in range(TPB):
            sld = sp.tile([128, CT], bf16, name="sld", tag="sld", bufs=2)
            nc.sync.dma_start(sld[:],
                              aps[f"scr_s_{bname}"][:, it * CT:(it + 1) * CT])
            prep = pp.tile([128, CT], f32, name="prep", tag="prep", bufs=2)
            mm(prep[:], BR("onescol32")[32:33, :],
               rAll[32:33, it * CT:(it + 1) * CT], True, True)
            t1 = sp.tile([128, CT], bf16, name="t1", tag="t1", bufs=2)
            nc.vector.tensor_mul(t1[:], sld[:], prep[:])
            hh4 = sp.tile([128, 2048], bf16, name="hh4", tag="hh4", bufs=2)
            for cp in range(2):
                pf = pp.tile([128, 1024], f32, name="pf", tag="pf", bufs=2)
                for ci in range(2):
                    c4 = cp * 2 + ci
                    mm(pf[:, ci * 512:(ci + 1) * 512],
                       BR(f"w1g{mi}")[:, c4 * 128:(c4 + 1) * 128], t1[:],
                       True, False)
                    mm(pf[:, ci * 512:(ci + 1) * 512], BR(f"gb1row{mi}{c4}"),
                       rAll[0:2, it * CT:(it + 1) * CT], False, True, kp=2)
                nc.scalar.activation(hh4[:, cp * 1024:(cp + 1) * 1024], pf[:],
                                     AF.Prelu, alpha=0.01)
            pf2 = pp.tile([128, CT], f32, name="pf2", tag="pf2", bufs=2)
            for c4 in range(4):
                mm(pf2[:], BR(f"ff2{mi}{c4}"),
                   hh4[:, c4 * 512:(c4 + 1) * 512], c4 == 0, False)
            mm(pf2[:], BR(f"af2row{mi}"),
               rAll[0:2, it * CT:(it + 1) * CT], False, True, kp=2)
            sft = sp.tile([128, CT], bf16, name="sft", tag="sft", bufs=2)
            nc.vector.scalar_tensor_tensor(sft[:], t1[:], VP(f"ang{mi}"),
                                           pf2[:], OP.mult, OP.add)
            nc.scalar.dma_start(
                aps["scr_sf"][:, bi * CBLK + it * CT:bi * CBLK + (it + 1) * CT],
                sft[:])
            tile_stats(sft[:], statF[bname], it, sp, pp, ("prep", 2), ("pf2", 2))
        rF, nmF = ln_finish(statF[bname], sp, f"F{bname}")
        load_rst_block(rF, nmF)
        for it in range(TPB):
            prepF = pp.tile([128, CT], f32, name="prepF", tag="prep", bufs=2)
            mm(prepF[:], BR("onescol32")[32:33, :],
               rAll[32:33, it * CT:(it + 1) * CT], True, True)
            sfl = sp.tile([128, CT], bf16, name="sfl", tag="sfl", bufs=2)
            nc.sync.dma_start(
                sfl[:],
                aps["scr_sf"][:, bi * CBLK + it * CT:bi * CBLK + (it + 1) * CT])
            t2 = sp.tile([128, CT], bf16, name="t2c", tag="t2c", bufs=2)
            nc.vector.tensor_mul(t2[:], sfl[:], prepF[:])
            pr2 = pp.tile([128, CT], f32, name="pr2", tag="pf2", bufs=2)
            mm(pr2[:], BR(f"flrow{mi}"),
               rAll[0:2, it * CT:(it + 1) * CT], True, True, kp=2)
            n2 = sp.tile([128, CT], f32, name="n2", tag="n2", bufs=2)
            nc.vector.scalar_tensor_tensor(n2[:], t2[:], VP(f"flg{mi}"),
                                           pr2[:], OP.mult, OP.add)
            nc.scalar.dma_start(
                aps["outp"][:, bi * CBLK + it * CT:bi * CBLK + (it + 1) * CT],
                n2[:])

    # ------------------------- schedule ------------------------------
    with tc.tile_pool(name="a_sb", bufs=1) as sp, \
            tc.tile_pool(name="a_wv", bufs=1) as wavep:
        with tc.tile_pool(name="a_ps", bufs=1, space="PSUM") as pp:
            for bname, mi, bi in [("n", 0, 0), ("l", 2, 3)]:
                for it in range(TPB):
                    phase_a_tile(bname, mi, bi, it, sp, pp, wavep)
            for it in range(TPB):
                mix_tile(it, sp, pp)
            for bname, mi, bi in [("t0", 1, 1), ("t1", 1, 2)]:
                for it in range(TPB):
                    phase_a_tile(bname, mi, bi, it, sp, pp, wavep)
        with tc.tile_pool(name="c_ps", bufs=1, space="PSUM") as pp2:
            for bname, mi, bi in BLOCKS:
                phase_c_block(bname, mi, bi, sp, pp2)


def _build_program(wp, vp, bp):
    nc = bacc.Bacc("TRN2", target_bir_lowering=False, debug=False,
                   num_devices=N_CORES)
    aps = {}
    aps["xpack"] = nc.dram_tensor("xpack", [128, 4 * TPB * TW], bf16,
                                  kind="ExternalInput").ap()
    aps["xqt"] = nc.dram_tensor("xqt", [128, 2 * CBLK], bf16,
                                kind="ExternalInput").ap()
    aps["vpack"] = nc.dram_tensor("vpack", [128, vp.n], f32,
                                  kind="ExternalInput").ap()
    aps["bpack"] = nc.dram_tensor("bpack", [128, bp.n], bf16,
                                  kind="ExternalInput").ap()
    aps["outp"] = nc.dram_tensor("outp", [128, 4 * CBLK], f32,
                                 kind="ExternalOutput").ap()
    for bname, _, _ in BLOCKS:
        aps[f"scr_s_{bname}"] = nc.dram_tensor(
            f"scr_s_{bname}", [128, CBLK], bf16).ap()
    for bname in ("n", "l"):
        aps[f"scr_ym_{bname}"] = nc.dram_tensor(
            f"scr_ym_{bname}", [128, CBLK], bf16).ap()
    aps["scr_sf"] = nc.dram_tensor("scr_sf", [128, 4 * CBLK], bf16).ap()

    with tile.TileContext(nc) as tc:
        with ExitStack() as ctx:
            _emit(ctx, tc, nc, aps, wp, vp, bp)
    nc.compile()
    return nc


_CACHE = {}


def kernel(**inputs):
    inputs = {k: np.asarray(v) for k, v in inputs.items()}
    wp, vp, bp = _host_pack(inputs)
    if "prog" not in _CACHE:
        _CACHE["prog"] = _build_program(wp, vp, bp)
    nc = _CACHE["prog"]
    vpack, bpack = vp.build(), bp.build()
    in_maps = []
    for b in range(B):
        xp = np.concatenate([
            _pack_x(inputs["x_node"][b]),
            _pack_x(inputs["x_trace"][b][:, 0:128]),
            _pack_x(inputs["x_trace"][b][:, 128:256]),
            _pack_x(inputs["x_log"][b]),
        ], axis=1)
        xqt = np.concatenate([_pack_x_qt(inputs["x_node"][b]),
                              _pack_x_qt(inputs["x_log"][b])], axis=1)
        in_maps.append({"xpack": np.ascontiguousarray(xp), "xqt": xqt,
                        "vpack": vpack, "bpack": bpack})
    res = run_bass_kernel_spmd(nc, in_maps, list(range(N_CORES)))
    out = np.empty((B, W, 2 * N + E, D), np.float32)
    for b in range(B):
        op = res.results[b]["outp"]                     # (128, 4*CBLK)
        for bi, j0 in [(0, 0), (1, N), (2, N + 128), (3, N + E)]:
            blk = op[:, bi * CBLK:(bi + 1) * CBLK]
            arr = blk.reshape(D, TPB, 8, W)             # (D, it, q, t)
            out[b, :, j0:j0 + 128, :] = arr.transpose(3, 1, 2, 0) \
                .reshape(W, 128, D)
    return out
